# revision 70
# baseline (speedup 1.0000x reference)
"""EnergyAE loss kernel for Trainium2 (Bass/Tile), 8-core data-parallel.

512-sample batch sharded 64/core; weights replicated. Returns the same
5-tuple as the reference: (neg_log_prob, recon_loss, latent_energy,
logdet_loss, sigma), each (512,) float32.

Per-core pipeline:
  S0  load x, PE-transpose to xT (D-on-partition); bias staging; bit patterns
  S1  h = tanh(x@W1+b1)  (x^T stationary, W1 streamed as k-strips)
  S2  [z*|log s] = [Wmu|Wls]^T h + bias; sigma; broadcasts; batch-layout z
  S3  decoder tanh features t,s=1-t^2,w=2ts at z*; V_aug=[diag(s)W1d^T | t]
  S4  stream W2 column-strips: J[dc]=W2[:,dc]^T V_aug (dec1 tail fused),
      packed 8-sample JTJ += J^T J, PE-transposed W2 blocks give g += W2 d^T
  S4b hess = W1d diag(2 t s g / sigma) W1d^T  (packed matmuls)
  S4c Prec_packed = JTJ*M_sigma + hess + I    (mask-matmul built operands)
  S4d unpack packed (128,(g,r,j)) -> per-sample (64, 256) via 64 tiny DMAs
  S6  Gershgorin bracket; Householder tridiagonalization; Sturm multisection
      eigmin; shift; Cholesky; U^-1; trace-inv; logdet; z_off = U^-1 eps
  S5  decoder at z_sample (W2 row-strips), recon loss, output assembly
"""
import numpy as np

import concourse.bass as bass
import concourse.tile as tile
from concourse import mybir

F32 = mybir.dt.float32
F32R = mybir.dt.float32r
BF16 = mybir.dt.bfloat16
F8E4 = mybir.dt.float8e4
F16 = mybir.dt.float16
MMPM = mybir.MatmulPerfMode
I32 = mybir.dt.int32
AX = mybir.AxisListType
ALU = mybir.AluOpType
ACTF = mybir.ActivationFunctionType
AP = bass.AP

D, H, N, BS = 3072, 2048, 16, 512
NCORES = 8
B = BS // NCORES            # 64
KC_H = H // 128             # 16
KC_D = D // 128             # 24
NGRP = B // 8               # 8
PACK = NGRP * 128           # 1024
BN = B * N                  # 1024
NSHIFT = 24                 # Sturm multisection grid
NSTURM = 2                  # multisection iterations


def _sap(t, offset, *dims):
    base = t[:]
    return AP(tensor=base.tensor, offset=base.offset + offset, ap=list(dims))


def split_excess_waits(nc, max_waits=1):
    """This walrus build accepts only one sync wait per instruction: move
    excess waits onto same-engine NoOps inserted just before."""
    n = 0
    for f in nc.m.functions:
        for bb in f.blocks:
            out = []
            for ins in bb.instructions:
                si = getattr(ins, "sync_info", None)
                ow = list(si.on_wait) if (si is not None and si.on_wait) else []
                if len(ow) > max_waits:
                    si.on_wait = ow[-max_waits:]
                    for w in ow[:-max_waits]:
                        n += 1
                        out.append(mybir.InstNoOp(
                            name=f"I-waitsplit-{n}",
                            sync_info=mybir.SyncInfo(on_wait=[w], on_update=[]),
                            bass_nofuse=True,
                            engine=ins.engine,
                        ))
                out.append(ins)
            bb.instructions = out
    return n


def build_module(debug=False):
    from contextlib import ExitStack

    nc = bass.Bass("TRN2", target_bir_lowering=False, debug=False,
                   num_devices=NCORES)

    x_d = nc.declare_dram_parameter("x", [B, D], F32R, isOutput=False)
    eps_d = nc.declare_dram_parameter("eps", [B, N], F32, isOutput=False)
    eW1_d = nc.declare_dram_parameter("enc_W1", [D, H], F16, isOutput=False)
    eb1_d = nc.declare_dram_parameter("enc_b1", [H], F32R, isOutput=False)
    eWmu_d = nc.declare_dram_parameter("enc_Wmu", [H, N], F32R, isOutput=False)
    ebmu_d = nc.declare_dram_parameter("enc_bmu", [N], F32R, isOutput=False)
    eWls_d = nc.declare_dram_parameter("enc_Wls", [H, 1], F32R, isOutput=False)
    ebls_d = nc.declare_dram_parameter("enc_bls", [1], F32R, isOutput=False)
    dW1_d = nc.declare_dram_parameter("dec_W1", [N, H], F32R, isOutput=False)
    db1_d = nc.declare_dram_parameter("dec_b1", [H], F32, isOutput=False)
    dW2_d = nc.declare_dram_parameter("dec_W2", [H, D], F8E4, isOutput=False)
    dW2T_d = nc.declare_dram_parameter("dec_W2T", [D, H], F8E4,
                                       isOutput=False)
    db2_d = nc.declare_dram_parameter("dec_b2", [D], F32R, isOutput=False)
    out_d = nc.declare_dram_parameter("out", [B, 5], F32, isOutput=True)

    dbg = {}
    if debug:
        for name, shape in [
            ("dbg_h", [B, H]), ("dbg_zsig", [B, N + 1]),
            ("dbg_t", [128, KC_H * B]), ("dbg_jtj", [128, PACK]),
            ("dbg_g", [128, KC_H * B]), ("dbg_dec1", [B, D]),
            ("dbg_hess", [128, PACK]), ("dbg_prec", [B, N * N]),
            ("dbg_tri", [B, 2 * N]), ("dbg_eig", [B, 4]),
            ("dbg_chol", [B, N * N]), ("dbg_xinv", [B, N * N]),
            ("dbg_zoff", [B, N]), ("dbg_parts", [B, 8]),
        ]:
            dbg[name] = nc.declare_dram_parameter(name, shape, F32,
                                                  isOutput=True)

    ctx = ExitStack()
    with tile.TileContext(nc) as tc, ctx:
        from contextlib import ExitStack as _ES
        per = ctx.enter_context(tc.tile_pool(name="per", bufs=1))
        dma2 = ctx.enter_context(tc.tile_pool(name="dma2", bufs=2))
        sm = ctx.enter_context(tc.tile_pool(name="sm", bufs=1))
        psctx = _ES()
        _pscur = [None]

        def psum_phase(name):
            nonlocal psctx
            psctx.close()
            psctx = _ES()
            _pscur[0] = psctx.enter_context(
                tc.tile_pool(name=name, bufs=1, space="PSUM"))
            return _pscur[0]
        V = nc.vector
        SC = nc.scalar

        def dbg_dump(name, src_ap, cast=False):
            if not debug:
                return
            nc.sync.dma_start(out=dbg[name][:],
                              in_=src_ap.bitcast(F32) if cast else src_ap)

        # ================= S0: inputs & patterns =================
        x_sb = per.tile([B, D], F32R, tag="Vbig")
        nc.sync.dma_start(out=x_sb, in_=x_d[:])
        eps_sb = per.tile([B, N], F32)
        nc.sync.dma_start(out=eps_sb, in_=eps_d[:])

        io_rowf = sm.tile([128, 128], F32)
        nc.gpsimd.iota(io_rowf[:], pattern=[[1, 128]], base=0,
                       channel_multiplier=0,
                       allow_small_or_imprecise_dtypes=True)
        pidx = sm.tile([128, 1], F32)
        nc.gpsimd.iota(pidx[:], pattern=[[0, 1]], base=0, channel_multiplier=1,
                       allow_small_or_imprecise_dtypes=True)
        ident = sm.tile([128, 128], F32R)
        V.tensor_scalar(out=ident[:], in0=io_rowf[:], scalar1=pidx[:],
                        scalar2=None, op0=ALU.is_equal)
        # bf16 identity: transposes with a bf16 moving operand cost 1.0
        # cycles/row instead of 1.5 (values are exact 0/1 in bf16)
        identb = sm.tile([128, 128], BF16)
        V.tensor_scalar(out=identb[:], in0=io_rowf[:], scalar1=pidx[:],
                        scalar2=None, op0=ALU.is_equal)
        ones_row = sm.tile([1, 128], F32R)
        V.tensor_scalar(out=ones_row[:], in0=io_rowf[0:1, :], scalar1=0.0,
                        scalar2=None, op0=ALU.is_ge)

        def pe_transpose(dst_ap, src_ap, p, f):
            pt = _pscur[0].tile([128, 128], F32R, name="pt_stage",
                                tag="pt_stage", bufs=2)
            nc.tensor.transpose(pt[:f, :p], src_ap, ident[:p, :p])
            V.tensor_copy(dst_ap, pt[:f, :p])

        psum_phase("ps0")

        xT = per.tile([128, KC_D, B], F32R)
        for dc in range(KC_D):
            pe_transpose(xT[:, dc, :], x_sb[:, dc * 128:(dc + 1) * 128], B, 128)

        db1r = sm.tile([1, H], F32R)
        nc.scalar.dma_start(out=db1r,
                            in_=AP(tensor=db1_d, offset=0,
                                   ap=[[0, 1], [1, H]]).bitcast(F32R))
        db1c = sm.tile([128, KC_H], F32)
        nc.sync.dma_start(out=db1c, in_=AP(tensor=db1_d, offset=0,
                                           ap=[[1, 128], [128, KC_H]]))

        muls = per.tile([128, KC_H, N + 1], F32R, tag="featF")
        nc.sync.dma_start(out=muls[:, :, 0:N],
                          in_=AP(tensor=eWmu_d, offset=0,
                                 ap=[[N, 128], [128 * N, KC_H], [1, N]]))
        nc.sync.dma_start(out=muls[:, :, N:N + 1],
                          in_=AP(tensor=eWls_d, offset=0,
                                 ap=[[1, 128], [128, KC_H], [0, 1]]))
        muls16 = per.tile([128, KC_H, N + 1], F16, tag="featF2")
        V.tensor_copy(muls16[:], muls[:].bitcast(F32))
        bmur = sm.tile([1, N + 1], F32R)
        nc.sync.dma_start(out=bmur[:, 0:N], in_=AP(tensor=ebmu_d, offset=0,
                                                   ap=[[0, 1], [1, N]]))
        nc.sync.dma_start(out=bmur[:, N:N + 1],
                          in_=AP(tensor=ebls_d, offset=0, ap=[[0, 1], [1, 1]]))
        # resident dec_W1: [N, kc, 128] in one DMA; transposed + bf16 copies
        w1dc_res = per.tile([N, KC_H, 128], F32R, tag="featD")
        nc.scalar.dma_start(out=w1dc_res,
                            in_=AP(tensor=dW1_d, offset=0,
                                   ap=[[H, N], [128, KC_H], [1, 128]]))
        w1dc_b16 = sm.tile([N, KC_H, 128], BF16)
        V.tensor_copy(w1dc_b16[:], w1dc_res[:].bitcast(F32))
        w1dT = per.tile([128, KC_H, N], F32R)
        for kc in range(KC_H):
            pe_transpose(w1dT[:, kc, :], w1dc_res[:, kc, :], N, 128)
        # dec_b2 in column layout [128, KC_D] (partition = d within strip)
        db2col = sm.tile([128, KC_D], F32R)
        nc.sync.dma_start(out=db2col, in_=AP(tensor=db2_d, offset=0,
                                             ap=[[1, 128], [128, KC_D]]))
        onescol_b = sm.tile([128, 1], BF16)
        V.tensor_scalar(out=onescol_b[:], in0=pidx[:], scalar1=-1.0,
                        scalar2=None, op0=ALU.is_gt)
        # resident fp8 dec_W2 as kc-pair tiles (DoubleRow lhsT needs the
        # pair dim inside one tile) and pre-transposed d-strips for g
        W2P8 = [per.tile([128, 2, D], F8E4, name=f"w2p{p}")
                for p in range(KC_H // 2)]
        W2TP8 = [per.tile([128, 2, KC_H * 128], F8E4, name=f"w2t{dp}")
                 for dp in range(KC_D // 2)]

        # ================= S1: encoder h =================
        ps = _pscur[0]
        ph = [ps.tile([B, 512], F32, name=f"ph{i}") for i in range(4)]
        for nck in range(4):
            eb1c = sm.tile([1, 512], F32R, name="eb1c", tag="b512")
            nc.scalar.dma_start(out=eb1c, in_=AP(tensor=eb1_d, offset=nck * 512,
                                               ap=[[0, 1], [1, 512]]))
            nc.tensor.matmul(ph[nck][:], ones_row[:, 0:B], eb1c[:],
                             start=True, stop=False)
        xT_f16 = per.tile([128, KC_D, B], F16)
        V.tensor_copy(xT_f16[:], xT[:].bitcast(F32))
        for kc in range(KC_D):
            for hf in range(2):
                w1s = dma2.tile([128, H // 2], F16, name="w1s",
                                tag="wstream", bufs=4)
                qeng = (nc.sync, nc.scalar, nc.gpsimd)[(2 * kc + hf) % 3]
                qeng.dma_start(
                    out=w1s, in_=eW1_d[kc * 128:(kc + 1) * 128,
                                       hf * 1024:(hf + 1) * 1024])
                for nk in range(2):
                    nck = hf * 2 + nk
                    nc.tensor.matmul(ph[nck][:], xT_f16[:, kc, :],
                                     w1s[:, nk * 512:(nk + 1) * 512],
                                     start=False, stop=(kc == KC_D - 1),
                                     skip_group_check=(kc != KC_D - 1))
        for p in range(KC_H // 2):
            (nc.sync if p % 2 == 0 else nc.scalar).dma_start(
                out=W2P8[p][:],
                in_=AP(tensor=dW2_d, offset=p * 256 * D,
                       ap=[[D, 128], [128 * D, 2], [1, D]]))
        for dp in range(KC_D // 2):
            (nc.sync if dp % 2 == 0 else nc.scalar).dma_start(
                out=W2TP8[dp][:],
                in_=AP(tensor=dW2T_d, offset=dp * 256 * H,
                       ap=[[H, 128], [128 * H, 2], [1, H]]))
        h_sb = per.tile([B, H], F32R, tag="Vbig")
        for nck in range(4):
            SC.activation(h_sb[:, nck * 512:(nck + 1) * 512], ph[nck][:],
                          ACTF.Tanh)
        dbg_dump("dbg_h", h_sb[:], cast=True)
        hT = per.tile([128, KC_H, B], F32R, tag="featD")
        hT16 = per.tile([128, KC_H, B], F16)
        for kc in range(KC_H):
            pe_transpose(hT[:, kc, :], h_sb[:, kc * 128:(kc + 1) * 128], B, 128)
        V.tensor_copy(hT16[:], hT[:].bitcast(F32))

        # ================= S2: z_star / sigma =================
        ps = psum_phase("ps2")
        pz = ps.tile([N, B], F32, name="pz")
        nc.tensor.matmul(pz[:], bmur[:, 0:N], ones_row[:, 0:B], start=True,
                         stop=False)
        for kc in range(KC_H):
            nc.tensor.matmul(pz[:], muls16[:, kc, 0:N], hT16[:, kc, :],
                             start=False, stop=(kc == KC_H - 1),
                             skip_group_check=(kc != KC_H - 1))
        pzs = ps.tile([1, B], F32, name="pzs")
        nc.tensor.matmul(pzs[:], bmur[:, N:N + 1], ones_row[:, 0:B],
                         start=True, stop=False)
        for kc in range(KC_H):
            nc.tensor.matmul(pzs[:], muls16[:, kc, N:N + 1], hT16[:, kc, :],
                             start=False, stop=(kc == KC_H - 1),
                             skip_group_check=(kc != KC_H - 1))
        zT = per.tile([N, B], F32R)
        V.tensor_copy(zT[:], pz[:])
        sig_row = sm.tile([1, B], F32R)
        SC.activation(sig_row[:], pzs[:], ACTF.Exp)
        invsigT = sm.tile([1, B], F32R)
        with nc.allow_low_precision(reason="fp32r bits are full fp32 here"):
            V.reciprocal(invsigT[:], sig_row[:].bitcast(F32))
        pb = ps.tile([128, B], F32, name="pb")
        nc.tensor.matmul(pb[:], ones_row[:, 0:128], invsigT[:],
                         start=True, stop=True)
        invsig_bc = per.tile([128, B], F32)
        V.tensor_copy(invsig_bc[:], pb[:])
        # batch layout via matmul transposes: zsig (B, 17)
        pzb = ps.tile([B, N], F32, name="pzb")
        nc.tensor.matmul(pzb[:], zT[:], ident[0:N, 0:N],
                         start=True, stop=True)
        psb = ps.tile([B, 64], F32, name="psb")
        nc.tensor.matmul(psb[:], sig_row[:], ones_row[:, 0:64],
                         start=True, stop=True)
        zsig = per.tile([B, N + 1], F32R)
        V.tensor_copy(zsig[:, 0:N], pzb[:])
        V.tensor_copy(zsig[:, N:N + 1], psb[:, 0:1])
        z_b = zsig[:, 0:N].bitcast(F32)
        sig_b = zsig[:, N:N + 1].bitcast(F32)
        dbg_dump("dbg_zsig", zsig[:], cast=True)
        invsig_b = sm.tile([B, 1], F32)
        V.reciprocal(invsig_b[:], sig_b)
        invsig2_b = sm.tile([B, 1], F32)
        V.tensor_tensor(out=invsig2_b[:], in0=invsig_b[:], in1=invsig_b[:],
                        op=ALU.mult)
        zT_b16 = sm.tile([N, B], BF16)
        V.tensor_copy(zT_b16[:], zT[:].bitcast(F32))
        neg_invsig_bc = per.tile([128, B], F32)
        V.tensor_scalar(out=neg_invsig_bc[:], in0=invsig_bc[:], scalar1=-1.0,
                        scalar2=None, op0=ALU.mult)

        # ================= S3: decoder features at z_star =================
        tT = per.tile([128, KC_H, B], BF16, tag="featB")
        sT = per.tile([128, KC_H, B], BF16, tag="featA")
        wT = per.tile([128, KC_H, B], F32, tag="featE")
        ps = psum_phase("ps3")
        for kq in range(4):
            pa = ps.tile([128, 4, B], F32, name="pa", tag="pa", bufs=2)
            for kk in range(4):
                kc = kq * 4 + kk
                nc.tensor.matmul(pa[:, kk, :],
                                 db1r[0:1, kc * 128:(kc + 1) * 128],
                                 ones_row[0:1, 0:B], start=True, stop=False)
                nc.tensor.matmul(pa[:, kk, :], w1dc_b16[:, kc, :], zT_b16[:],
                                 start=False, stop=True)
            SC.activation(tT[:, kq * 4:(kq + 1) * 4, :], pa[:], ACTF.Tanh)
            t2f = sm.tile([128, 4, B], F32, name="t2f", tag="t2f", bufs=2)
            SC.activation(t2f[:], tT[:, kq * 4:(kq + 1) * 4, :], ACTF.Square)
            V.tensor_scalar(out=sT[:, kq * 4:(kq + 1) * 4, :], in0=t2f[:],
                            scalar1=-1.0, scalar2=1.0, op0=ALU.mult,
                            op1=ALU.add)
            V.scalar_tensor_tensor(out=wT[:, kq * 4:(kq + 1) * 4, :],
                                   in0=tT[:, kq * 4:(kq + 1) * 4, :],
                                   scalar=2.0,
                                   in1=sT[:, kq * 4:(kq + 1) * 4, :],
                                   op0=ALU.mult, op1=ALU.mult)

        tT8 = per.tile([128, KC_H, B], F8E4)
        V.tensor_copy(tT8[:], tT[:])
        Vaug = per.tile([128, KC_H, BN], F8E4, tag="Vbig")
        vp = Vaug[:].ap[0][0]
        sp_ = sT[:].ap[0][0]
        wtp = w1dT[:].ap[0][0]
        for kc in range(KC_H):
            eng = V if kc % 2 == 0 else nc.gpsimd
            eng.tensor_tensor(
                out=_sap(Vaug, kc * BN, [vp, 128], [N, B], [1, N]),
                in0=_sap(sT, kc * B, [sp_, 128], [1, B], [0, N]),
                in1=_sap(w1dT, kc * N, [wtp, 128], [0, B], [1, N]).bitcast(F32),
                op=ALU.mult)

        # ===== S4: fused W2 single-pass loop =====
        # per d-strip: DMA fp32 strip -> Pool cast to resident bf16 ->
        # dec1T matmuls -> dT -> W2T transposes -> J strip -> g -> JTJ
        dT_all = per.tile([128, KC_D, B], F8E4)
        ps = psum_phase("ps4")
        Jsb2 = sm.tile([128, 2, BN], F8E4, name="Jsb2")
        pJlo = ps.tile([128, 512], F32, name="pJlo")           # 1 bank
        pJhi = ps.tile([128, 512], F32, name="pJhi")           # 1 bank
        pJTJ = ps.tile([128, NGRP, 128], F32, name="pJTJ")     # 2 banks
        pg = ps.tile([128, KC_H, B], F32, name="pgall")        # 2 banks
        for dc in range(KC_D):
            pdec = ps.tile([128, B], F32, name="pdec", tag="pdec")
            for p in range(KC_H // 2):
                nc.tensor.matmul(pdec[:],
                                 W2P8[p][:, :, dc * 128:(dc + 1) * 128],
                                 tT8[:, 2 * p:2 * p + 2, :],
                                 start=(p == 0), stop=(p == KC_H // 2 - 1),
                                 skip_group_check=(p not in
                                                   (0, KC_H // 2 - 1)),
                                 perf_mode=MMPM.DoubleRow)
            dfc = sm.tile([128, B], F32, name="dfc", tag="diff", bufs=2)
            V.scalar_tensor_tensor(out=dfc[:], in0=pdec[:],
                                   scalar=db2col[:, dc:dc + 1].bitcast(F32),
                                   in1=xT[:, dc, :].bitcast(F32),
                                   op0=ALU.add, op1=ALU.subtract)
            V.tensor_tensor(out=dT_all[:, dc, :], in0=dfc[:],
                            in1=neg_invsig_bc[:], op=ALU.mult)
            vap = Vaug[:].ap[0][0]
            NPAIR = KC_H // 2
            for pr in range(NPAIR):
                nc.tensor.matmul(
                    pJlo[:],
                    W2P8[pr][:, :, dc * 128:(dc + 1) * 128],
                    _sap(Vaug, (2 * pr) * BN, [vap, 128], [BN, 2], [1, 512]),
                    start=(pr == 0), stop=(pr == NPAIR - 1),
                    skip_group_check=(pr not in (0, NPAIR - 1)),
                    perf_mode=MMPM.DoubleRow)
            V.tensor_copy(Jsb2[:, dc % 2, 0:512], pJlo[:])
            for pr in range(NPAIR):
                nc.tensor.matmul(
                    pJhi[:],
                    W2P8[pr][:, :, dc * 128:(dc + 1) * 128],
                    _sap(Vaug, (2 * pr) * BN + 512, [vap, 128], [BN, 2],
                         [1, 512]),
                    start=(pr == 0), stop=(pr == NPAIR - 1),
                    skip_group_check=(pr not in (0, NPAIR - 1)),
                    perf_mode=MMPM.DoubleRow)
            V.tensor_copy(Jsb2[:, dc % 2, 512:1024], pJhi[:])
            if dc % 2 == 1:
                dp = dc // 2
                for kc in range(KC_H):
                    st = (dp == 0 and kc in (0, 8))
                    sp = (dp == KC_D // 2 - 1 and kc in (7, 15))
                    nc.tensor.matmul(pg[:, kc, :],
                                     W2TP8[dp][:, :,
                                               kc * 128:(kc + 1) * 128],
                                     dT_all[:, dc - 1:dc + 1, :],
                                     start=st, stop=sp,
                                     skip_group_check=not (st or sp),
                                     perf_mode=MMPM.DoubleRow)
            if dc % 2 == 1:
                dp = dc // 2
                for g in range(NGRP):
                    st = (dp == 0 and g in (0, 4))
                    sp = (dp == KC_D // 2 - 1 and g in (3, 7))
                    nc.tensor.matmul(pJTJ[:, g, :],
                                     Jsb2[:, :, g * 128:(g + 1) * 128],
                                     Jsb2[:, :, g * 128:(g + 1) * 128],
                                     start=st, stop=sp,
                                     skip_group_check=not (st or sp),
                                     perf_mode=MMPM.DoubleRow)
        JTJsb = per.tile([128, PACK], F32, tag="featD")
        V.tensor_copy(JTJsb[:], pJTJ[:].rearrange("p a b -> p (a b)"))
        gsb = per.tile([128, KC_H, B], BF16, tag="featC")
        V.tensor_tensor(out=gsb[:], in0=pg[:],
                        in1=_sap(invsig_bc, 0, [invsig_bc[:].ap[0][0], 128],
                                 [0, KC_H], [1, B]),
                        op=ALU.mult)
        dbg_dump("dbg_jtj", JTJsb[:])
        dbg_dump("dbg_g", gsb[:].rearrange("p a b -> p (a b)"))

        # ================= S4b: hess =================
        w1rep = per.tile([128, KC_H, 128], F8E4, tag="featF")
        for kc in range(KC_H):
            V.tensor_copy(w1rep[:, kc, :],
                          _sap(w1dT, kc * N, [wtp, 128], [0, 8],
                               [1, N]).bitcast(F32))
        cT = per.tile([128, KC_H, B], F32, tag="featB")
        V.tensor_tensor(out=cT[:], in0=wT[:], in1=gsb[:], op=ALU.mult)
        Vc = per.tile([128, KC_H, BN], F8E4, tag="Vbig")
        cp_ = cT[:].ap[0][0]
        for kc in range(KC_H):
            eng = V if kc % 2 == 0 else nc.gpsimd
            eng.tensor_tensor(
                out=_sap(Vc, kc * BN, [Vc[:].ap[0][0], 128], [N, B], [1, N]),
                in0=_sap(cT, kc * B, [cp_, 128], [1, B], [0, N]),
                in1=_sap(w1dT, kc * N, [wtp, 128], [0, B], [1, N]).bitcast(F32),
                op=ALU.mult)
        ps = psum_phase("ps4b")
        pH = ps.tile([128, NGRP, 128], F32, name="pH")
        for p in range(KC_H // 2):
            for g in range(NGRP):
                st = (p == 0 and g in (0, 4))
                sp = (p == KC_H // 2 - 1 and g in (3, 7))
                nc.tensor.matmul(
                    pH[:, g, :],
                    _sap(Vc, (2 * p) * BN + g * 128,
                         [Vc[:].ap[0][0], 128], [BN, 2], [1, 128]),
                    w1rep[:, 2 * p:2 * p + 2, :], start=st, stop=sp,
                    skip_group_check=not (st or sp),
                    perf_mode=MMPM.DoubleRow)
        hesssb = per.tile([128, PACK], F32, tag="featE")
        V.tensor_copy(hesssb[:], pH[:].rearrange("p a b -> p (a b)"))
        dbg_dump("dbg_hess", hesssb[:])

        # ================= S4c: Prec_packed =================
        # per-(row r, group g) scale 1/sigma^2(g*8+r) built by one mask
        # matmul; cross-sample blocks stay garbage (never read by unpack)
        ia_rf = sm.tile([B, 128], F32)
        nc.gpsimd.iota(ia_rf[:], pattern=[[1, 8], [0, 16]], base=0,
                       channel_multiplier=0,
                       allow_small_or_imprecise_dtypes=True)
        ibf = sm.tile([B, 1], F32)
        nc.gpsimd.iota(ibf[:], pattern=[[0, 1]], base=0, channel_multiplier=1,
                       allow_small_or_imprecise_dtypes=True)
        ibgf = sm.tile([B, 1], F32)
        V.memset(ibgf[:], 0.0)
        for kq in range(1, 8):
            V.scalar_tensor_tensor(out=ibgf[:], in0=ibf[:],
                                   scalar=float(8 * kq), in1=ibgf[:],
                                   op0=ALU.is_ge, op1=ALU.add)
        ib7f = sm.tile([B, 1], F32)
        V.tensor_scalar(out=ib7f[:], in0=ibgf[:], scalar1=-8.0, scalar2=None,
                        op0=ALU.mult)
        V.tensor_tensor(out=ib7f[:], in0=ibf[:], in1=ib7f[:], op=ALU.add)
        E2 = sm.tile([B, 128], F32R)
        V.tensor_scalar(out=E2[:], in0=ia_rf[:], scalar1=ib7f[:],
                        scalar2=None, op0=ALU.is_equal)
        ig8 = sm.tile([B, 8], F32)
        nc.gpsimd.iota(ig8[:], pattern=[[1, 8]], base=0, channel_multiplier=0,
                       allow_small_or_imprecise_dtypes=True)
        R2g = sm.tile([B, 8], F32R)
        V.tensor_scalar(out=R2g[:], in0=ig8[:], scalar1=ibgf[:],
                        scalar2=None, op0=ALU.is_equal)
        V.tensor_scalar(out=R2g[:], in0=R2g[:].bitcast(F32),
                        scalar1=invsig2_b[:], scalar2=None, op0=ALU.mult)
        ps2g = ps.tile([128, 8], F32, name="ps2g")
        nc.tensor.matmul(ps2g[:], E2[:], R2g[:], start=True, stop=True)
        s2g = sm.tile([128, 8], F32)
        V.tensor_copy(s2g[:], ps2g[:])
        prec_pack = JTJsb
        for g in range(NGRP):
            V.tensor_scalar(out=prec_pack[:, g * 128:(g + 1) * 128],
                            in0=prec_pack[:, g * 128:(g + 1) * 128],
                            scalar1=s2g[:, g:g + 1], scalar2=None,
                            op0=ALU.mult)
        V.tensor_tensor(out=prec_pack[:], in0=prec_pack[:], in1=hesssb[:],
                        op=ALU.add)

        # ================= S4d: unpack =================
        prec = per.tile([B, N * N], F32)
        ppp = prec_pack[:].ap[0][0]
        pp_out = prec[:].ap[0][0]
        # partition<->sample shuffle bounces through DRAM: SBUF DMAs allow
        # partition steps only in leading dims, DRAM side is unconstrained
        uscr = nc.dram_tensor("unpack_scr", [B, N * N], F32)
        for r in range(8):
            (nc.sync if r % 2 == 0 else nc.scalar).dma_start(
                out=AP(tensor=uscr, offset=r * 256,
                       ap=[[16, 16], [8 * 256, NGRP], [1, 16]]),
                in_=_sap(prec_pack, r * 16 * ppp + r * 16,
                         [ppp, 16], [128, NGRP], [1, 16]))
        nc.sync.dma_start(out=prec[:], in_=uscr[:])
        dbg_dump("dbg_prec", prec[:])

        # ================= S6: eigmin =================
        pcp = prec[:].ap[0][0]

        def pdiag(t, stride=N + 1, n=N, offset=0):
            return _sap(t, offset, [t[:].ap[0][0], B], [stride, n])

        V.tensor_scalar(out=pdiag(prec), in0=pdiag(prec), scalar1=1.0,
                        scalar2=None, op0=ALU.add)
        absr = sm.tile([B, N], F32)
        V.tensor_reduce(out=absr[:],
                        in_=prec[:].rearrange("b (i j) -> b i j", i=N),
                        axis=AX.X, op=ALU.add, apply_absolute_value=True)
        dg = sm.tile([B, N], F32)
        V.tensor_copy(dg[:], pdiag(prec))
        lo_s = sm.tile([B, 1], F32)
        hi_s = sm.tile([B, 1], F32)
        lo_v = sm.tile([B, N], F32)
        V.tensor_scalar(out=lo_v[:], in0=dg[:], scalar1=2.0, scalar2=None,
                        op0=ALU.mult)
        V.tensor_tensor(out=lo_v[:], in0=lo_v[:], in1=absr[:], op=ALU.subtract)
        V.tensor_reduce(out=lo_s[:], in_=lo_v[:], axis=AX.X, op=ALU.min)
        V.tensor_reduce(out=hi_s[:], in_=dg[:], axis=AX.X, op=ALU.min)

        # --- Householder tridiagonalization ---
        A2 = per.tile([B, N * N], F32)
        V.tensor_copy(A2[:], prec[:])
        Ed = sm.tile([B, N], F32)
        V.memset(Ed[:], 0.0)
        ap2 = A2[:].ap[0][0]
        vvt = sm.tile([B, N], F32, name="vvt")
        vstep = vvt[:].ap[0][0]
        tmpm = sm.tile([B, N], F32, name="tmpm")
        qvt = sm.tile([B, N], F32, name="qvt")
        qstep = qvt[:].ap[0][0]
        omm = sm.tile([B, N * N], F32, name="omm", tag="esolv")
        s1 = sm.tile([B, 1], F32, name="s1t")
        s2 = sm.tile([B, 1], F32, name="s2t")
        s3 = sm.tile([B, 1], F32, name="s3t")
        s4 = sm.tile([B, 1], F32, name="s4t")
        for k in range(N - 2):
            m = N - 1 - k
            xap = _sap(A2, (k + 1) * N + k, [ap2, B], [N, m])
            vt = vvt[:, 0:m]
            V.tensor_copy(vt, xap)
            V.tensor_tensor(out=tmpm[:, 0:m], in0=vt, in1=vt, op=ALU.mult)
            V.tensor_reduce(out=s1[:], in_=tmpm[:, 0:m], axis=AX.X, op=ALU.add)
            SC.activation(s2[:], s1[:], ACTF.Sqrt)
            V.scalar_tensor_tensor(out=s3[:], in0=vt[:, 0:1], scalar=0.0,
                                   in1=s2[:], op0=ALU.is_ge, op1=ALU.mult)
            edk = Ed[:, k + 1:k + 2]
            V.scalar_tensor_tensor(out=edk, in0=s3[:], scalar=-2.0,
                                   in1=s2[:], op0=ALU.mult, op1=ALU.add)
            # ||v'||^2 = 2*(s1 - v0*s3) algebraically (s3^2 == s1)
            V.tensor_tensor(out=s4[:], in0=vt[:, 0:1], in1=edk, op=ALU.mult)
            V.tensor_tensor(out=s4[:], in0=s1[:], in1=s4[:], op=ALU.subtract)
            V.tensor_scalar(out=s4[:], in0=s4[:], scalar1=2.0, scalar2=1e-30,
                            op0=ALU.mult, op1=ALU.max)
            V.tensor_tensor(out=vt[:, 0:1], in0=vt[:, 0:1], in1=edk,
                            op=ALU.subtract)
            V.reciprocal(s2[:], s4[:])    # 1/||v||^2 == beta/2
            asub = _sap(A2, (k + 1) * (N + 1), [ap2, B], [N, m], [1, m])
            V.tensor_tensor(
                out=omm[:, 0:m * m].rearrange("b (i j) -> b i j", i=m),
                in0=asub,
                in1=_sap(vvt, 0, [vstep, B], [0, m], [1, m]),
                op=ALU.mult)
            pvec = tmpm[:, 0:m]
            V.tensor_reduce(out=pvec,
                            in_=omm[:, 0:m * m].rearrange("b (i j) -> b i j",
                                                          i=m),
                            axis=AX.X, op=ALU.add)
            V.tensor_tensor(out=qvt[:, 0:m], in0=pvec, in1=vt, op=ALU.mult)
            V.tensor_reduce(out=s1[:], in_=qvt[:, 0:m], axis=AX.X, op=ALU.add)
            V.tensor_tensor(out=s1[:], in0=s1[:], in1=s2[:], op=ALU.mult)
            V.tensor_scalar(out=qvt[:, 0:m], in0=vt, scalar1=s1[:],
                            scalar2=None, op0=ALU.mult)
            V.tensor_tensor(out=qvt[:, 0:m], in0=pvec, in1=qvt[:, 0:m],
                            op=ALU.subtract)
            V.tensor_scalar(out=s4[:], in0=s2[:], scalar1=-2.0, scalar2=None,
                            op0=ALU.mult)    # -beta
            V.tensor_tensor(
                out=omm[:, 0:m * m].rearrange("b (i j) -> b i j", i=m),
                in0=_sap(vvt, 0, [vstep, B], [1, m], [0, m]),
                in1=_sap(qvt, 0, [qstep, B], [0, m], [1, m]),
                op=ALU.mult)
            V.scalar_tensor_tensor(
                out=asub,
                in0=omm[:, 0:m * m].rearrange("b (i j) -> b i j", i=m),
                scalar=s4[:], in1=asub, op0=ALU.mult, op1=ALU.add)
            V.tensor_tensor(
                out=omm[:, 0:m * m].rearrange("b (i j) -> b i j", i=m),
                in0=_sap(qvt, 0, [qstep, B], [1, m], [0, m]),
                in1=_sap(vvt, 0, [vstep, B], [0, m], [1, m]),
                op=ALU.mult)
            V.scalar_tensor_tensor(
                out=asub,
                in0=omm[:, 0:m * m].rearrange("b (i j) -> b i j", i=m),
                scalar=s4[:], in1=asub, op0=ALU.mult, op1=ALU.add)
        Td = sm.tile([B, N], F32)
        V.tensor_copy(Td[:], pdiag(A2))
        nege2 = sm.tile([B, N], F32)
        V.tensor_tensor(out=nege2[:], in0=Ed[:], in1=Ed[:], op=ALU.mult)
        V.tensor_scalar(out=nege2[:], in0=nege2[:], scalar1=-1.0,
                        scalar2=-1e-30, op0=ALU.mult, op1=ALU.add)
        if debug:
            tri = sm.tile([B, 2 * N], F32, name="dbtri")
            V.tensor_copy(tri[:, 0:N], Td[:])
            V.tensor_copy(tri[:, N:2 * N], Ed[:])
            nc.sync.dma_start(out=dbg["dbg_tri"][:], in_=tri[:])

        # --- Sturm multisection ---
        iotaF = sm.tile([B, NSHIFT], F32)
        ioi2 = sm.tile([B, NSHIFT], I32)
        nc.gpsimd.iota(ioi2[:], pattern=[[1, NSHIFT]], base=1,
                       channel_multiplier=0)
        V.tensor_copy(iotaF[:], ioi2[:])
        wid = sm.tile([B, 1], F32)
        V.tensor_tensor(out=wid[:], in0=hi_s[:], in1=lo_s[:], op=ALU.subtract)
        grid = sm.tile([B, NSHIFT], F32)
        dxm = sm.tile([B, N, NSHIFT], F32, tag="scr4k_a")
        pph = sm.tile([B, NSHIFT, N], F32)
        rr = sm.tile([B, NSHIFT], F32)
        cnt = sm.tile([B, NSHIFT], F32)
        stp = sm.tile([B, 1], F32)
        for it in range(NSTURM):
            V.tensor_scalar(out=stp[:], in0=wid[:],
                            scalar1=1.0 / (NSHIFT + 1.0), scalar2=None,
                            op0=ALU.mult)
            V.tensor_scalar(out=grid[:], in0=iotaF[:], scalar1=stp[:],
                            scalar2=lo_s[:], op0=ALU.mult, op1=ALU.add)
            V.tensor_tensor(out=dxm[:],
                            in0=_sap(Td, 0, [Td[:].ap[0][0], B], [1, N],
                                     [0, NSHIFT]),
                            in1=_sap(grid, 0, [grid[:].ap[0][0], B], [0, N],
                                     [1, NSHIFT]),
                            op=ALU.subtract)
            php = pph[:].ap[0][0]
            V.tensor_copy(_sap(pph, 0, [php, B], [N, NSHIFT]), dxm[:, 0, :])
            for i in range(1, N):
                V.reciprocal(rr[:], _sap(pph, i - 1, [php, B], [N, NSHIFT]))
                V.scalar_tensor_tensor(out=_sap(pph, i, [php, B], [N, NSHIFT]),
                                       in0=rr[:],
                                       scalar=nege2[:, i:i + 1],
                                       in1=dxm[:, i, :], op0=ALU.mult,
                                       op1=ALU.add)
            V.tensor_scalar(out=pph[:], in0=pph[:], scalar1=1e-25,
                            scalar2=None, op0=ALU.is_lt)
            V.tensor_reduce(out=cnt[:], in_=pph[:],
                            axis=AX.X, op=ALU.add)
            V.tensor_scalar(out=rr[:], in0=cnt[:], scalar1=0.0, scalar2=None,
                            op0=ALU.is_equal)
            V.tensor_reduce(out=s1[:], in_=rr[:], axis=AX.X, op=ALU.add)
            V.scalar_tensor_tensor(out=lo_s[:], in0=s1[:], scalar=stp[:],
                                   in1=lo_s[:], op0=ALU.mult, op1=ALU.add)
            V.tensor_copy(wid[:], stp[:])
        eigmin = sm.tile([B, 1], F32)
        V.tensor_scalar(out=eigmin[:], in0=wid[:], scalar1=0.5,
                        scalar2=None, op0=ALU.mult)
        V.tensor_tensor(out=eigmin[:], in0=lo_s[:], in1=eigmin[:], op=ALU.add)
        delta = sm.tile([B, 1], F32)
        V.tensor_scalar(out=delta[:], in0=eigmin[:], scalar1=-1.0,
                        scalar2=10.0, op0=ALU.mult, op1=ALU.add)
        if debug:
            de = sm.tile([B, 4], F32, name="dbeig")
            V.tensor_copy(de[:, 0:1], eigmin[:])
            V.tensor_copy(de[:, 1:2], delta[:])
            V.tensor_copy(de[:, 2:3], lo_s[:])
            V.tensor_copy(de[:, 3:4], hi_s[:])
            nc.sync.dma_start(out=dbg["dbg_eig"][:], in_=de[:])

        # ============ S6b: LDL^T of Prec + delta*I (sqrt-free) ============
        U = A2  # reuse A2 storage: overwrite with a fresh copy of prec
        V.tensor_copy(U[:], prec[:])
        V.tensor_scalar(out=pdiag(U), in0=pdiag(U), scalar1=delta[:],
                        scalar2=None, op0=ALU.add)
        dvec = sm.tile([B, N], F32)   # pivots d_k
        rvec = sm.tile([B, N], F32)   # 1/d_k
        for k in range(N):
            m = N - 1 - k
            dkk = _sap(U, k * (N + 1), [ap2, B], [1, 1])
            V.reciprocal(rvec[:, k:k + 1], dkk)
            if m > 0:
                urow = _sap(U, k * N + k + 1, [ap2, B], [1, m])
                V.tensor_scalar(out=vvt[:, 0:m], in0=urow, scalar1=-1.0,
                                scalar2=None, op0=ALU.mult)   # -a
                V.tensor_scalar(out=urow, in0=urow,
                                scalar1=rvec[:, k:k + 1], scalar2=None,
                                op0=ALU.mult)                 # l = a/d
                sub = _sap(U, (k + 1) * (N + 1), [ap2, B], [N, m], [1, m])
                V.tensor_tensor(
                    out=omm[:, 0:m * m].rearrange("b (i j) -> b i j", i=m),
                    in0=_sap(vvt, 0, [vstep, B], [1, m], [0, m]),
                    in1=_sap(U, k * N + k + 1, [ap2, B], [0, m], [1, m]),
                    op=ALU.mult)                  # (-a_i) * l_j
                V.tensor_tensor(
                    out=sub, in0=sub,
                    in1=omm[:, 0:m * m].rearrange("b (i j) -> b i j", i=m),
                    op=ALU.add)
        V.tensor_copy(dvec[:], pdiag(U))
        if debug:
            dbg_dump("dbg_chol", U[:])
        # logdet_loss = 0.5 * sum log d_k
        lud = sm.tile([B, N], F32)
        logdet = sm.tile([B, 1], F32)
        SC.activation(lud[:], dvec[:], ACTF.Ln, accum_out=logdet[:])
        V.tensor_scalar(out=logdet[:], in0=logdet[:], scalar1=0.5,
                        scalar2=None, op0=ALU.mult)

        # ========= S6c: M = (L^T)^{-1} (unit diag; XT[c,j] = M[j,c]) ======
        XT = per.tile([B, N * N], F32)
        V.memset(XT[:], 0.0)
        xtp = XT[:].ap[0][0]
        for k in range(N - 1, -1, -1):
            m = N - 1 - k
            if m > 0:
                V.tensor_tensor(
                    out=omm[:, 0:N * m].rearrange("b (c j) -> b c j", c=N),
                    in0=_sap(XT, k + 1, [xtp, B], [N, N], [1, m]),
                    in1=_sap(U, k * N + k + 1, [ap2, B], [0, N], [1, m]),
                    op=ALU.mult)
                V.tensor_reduce(
                    out=tmpm[:, 0:N],
                    in_=omm[:, 0:N * m].rearrange("b (c j) -> b c j", c=N),
                    axis=AX.X, op=ALU.add)
                V.tensor_scalar(out=_sap(XT, k, [xtp, B], [N, N]),
                                in0=tmpm[:, 0:N], scalar1=-1.0,
                                scalar2=None, op0=ALU.mult)
            V.tensor_scalar(out=_sap(XT, k * N + k, [xtp, B], [1, 1]),
                            in0=_sap(XT, k * N + k, [xtp, B], [1, 1]),
                            scalar1=1.0, scalar2=None, op0=ALU.add)
        if debug:
            dbg_dump("dbg_xinv", XT[:])
        # trinv = sum_c (sum_j M[j,c]^2) / d_c ; z_off = M.T... = U^-1 eps
        msq = sm.tile([B, N * N], F32, name="xsq", tag="esolv")
        V.tensor_tensor(out=msq[:], in0=XT[:], in1=XT[:], op=ALU.mult)
        V.tensor_reduce(out=tmpm[:, 0:N],
                        in_=msq[:].rearrange("b (c j) -> b c j", c=N),
                        axis=AX.X, op=ALU.add)
        trinv = sm.tile([B, 1], F32)
        V.tensor_tensor(out=qvt[:, 0:N], in0=tmpm[:, 0:N], in1=rvec[:],
                        op=ALU.mult)
        V.tensor_reduce(out=trinv[:], in_=qvt[:, 0:N], axis=AX.X, op=ALU.add)
        sqd = sm.tile([B, N], F32)
        SC.activation(sqd[:], rvec[:], ACTF.Sqrt)   # d^{-1/2}
        eh = sm.tile([B, N], F32)
        V.tensor_tensor(out=eh[:], in0=eps_sb[:], in1=sqd[:], op=ALU.mult)
        zoffm = sm.tile([B, N, N], F32, name="zoffm", tag="esolv")
        V.tensor_tensor(out=zoffm[:],
                        in0=_sap(XT, 0, [xtp, B], [1, N], [N, N]),
                        in1=_sap(eh, 0, [eh[:].ap[0][0], B], [0, N],
                                 [1, N]),
                        op=ALU.mult)
        z_off = sm.tile([B, N], F32)
        V.tensor_reduce(out=z_off[:], in_=zoffm[:], axis=AX.X, op=ALU.add)
        dbg_dump("dbg_zoff", z_off[:])
        z_samp = per.tile([B, N], F32R)
        V.tensor_tensor(out=z_samp[:], in0=z_b, in1=z_off[:], op=ALU.add)

        # latent_energy = 0.5*(|z*|^2 + trinv)
        zsq = sm.tile([B, N], F32, name="zsq")
        zn = sm.tile([B, 1], F32)
        SC.activation(zsq[:], z_b, ACTF.Square, accum_out=zn[:])
        lat = sm.tile([B, 1], F32)
        V.tensor_tensor(out=lat[:], in0=zn[:], in1=trinv[:], op=ALU.add)
        V.tensor_scalar(out=lat[:], in0=lat[:], scalar1=0.5, scalar2=None,
                        op0=ALU.mult)

        # ================= S5: recon at z_sample (d-layout) =================
        ps = psum_phase("ps5")
        zsT = per.tile([N, B], F32R)
        pe_transpose(zsT[:], z_samp[:], B, N)
        zsT_b16 = sm.tile([N, B], BF16)
        V.tensor_copy(zsT_b16[:], zsT[:].bitcast(F32))
        t2T = per.tile([128, KC_H, B], F8E4, tag="featA")
        for kq in range(4):
            pa2 = ps.tile([128, 4, B], F32, name="pa2", tag="pa2", bufs=2)
            for kk in range(4):
                kc = kq * 4 + kk
                nc.tensor.matmul(pa2[:, kk, :],
                                 db1r[0:1, kc * 128:(kc + 1) * 128],
                                 ones_row[0:1, 0:B], start=True, stop=False)
                nc.tensor.matmul(pa2[:, kk, :], w1dc_b16[:, kc, :],
                                 zsT_b16[:], start=False, stop=True)
            SC.activation(t2T[:, kq * 4:(kq + 1) * 4, :], pa2[:], ACTF.Tanh)
        pr22 = ps.tile([B, B], F32, name="pr22")
        for dc in range(KC_D):
            prc = ps.tile([128, B], F32, name="prc", tag="prc", bufs=2)
            for p in range(KC_H // 2):
                nc.tensor.matmul(prc[:],
                                 W2P8[p][:, :, dc * 128:(dc + 1) * 128],
                                 t2T[:, 2 * p:2 * p + 2, :],
                                 start=(p == 0), stop=(p == KC_H // 2 - 1),
                                 skip_group_check=(p not in
                                                   (0, KC_H // 2 - 1)),
                                 perf_mode=MMPM.DoubleRow)
            dfr = sm.tile([128, B], BF16, name="dfr", tag="diff", bufs=2)
            V.scalar_tensor_tensor(out=dfr[:], in0=prc[:],
                                   scalar=db2col[:, dc:dc + 1].bitcast(F32),
                                   in1=xT[:, dc, :].bitcast(F32),
                                   op0=ALU.add, op1=ALU.subtract)
            nc.tensor.matmul(pr22[:], dfr[:], dfr[:],
                             start=(dc == 0), stop=(dc == KC_D - 1),
                             skip_group_check=(dc not in (0, KC_D - 1)))
        dsq = sm.tile([B, B], F32)
        V.tensor_tensor(out=dsq[:], in0=pr22[:],
                        in1=ident[0:B, 0:B].bitcast(F32), op=ALU.mult)
        r2 = sm.tile([B, 1], F32)
        V.tensor_reduce(out=r2[:], in_=dsq[:], axis=AX.X, op=ALU.add)
        recon = sm.tile([B, 1], F32)
        V.scalar_tensor_tensor(out=recon[:], in0=r2[:], scalar=0.5,
                               in1=invsig2_b[:], op0=ALU.mult, op1=ALU.mult)

        # ================= outputs =================
        lsig = sm.tile([B, 1], F32)
        SC.activation(lsig[:], sig_b, ACTF.Ln)
        nlp = sm.tile([B, 1], F32)
        V.tensor_tensor(out=nlp[:], in0=recon[:], in1=lat[:], op=ALU.add)
        V.tensor_tensor(out=nlp[:], in0=nlp[:], in1=logdet[:], op=ALU.add)
        V.tensor_scalar(out=s1[:], in0=lsig[:], scalar1=float(D), scalar2=None,
                        op0=ALU.mult)
        V.tensor_tensor(out=nlp[:], in0=nlp[:], in1=s1[:], op=ALU.add)
        V.tensor_scalar(out=nlp[:], in0=nlp[:], scalar1=1.0 / D, scalar2=None,
                        op0=ALU.mult)
        outt = sm.tile([B, 5], F32)
        V.tensor_copy(outt[:, 0:1], nlp[:])
        V.tensor_copy(outt[:, 1:2], recon[:])
        V.tensor_copy(outt[:, 2:3], lat[:])
        V.tensor_copy(outt[:, 3:4], logdet[:])
        V.tensor_copy(outt[:, 4:5], sig_b)
        nc.sync.dma_start(out=out_d[:], in_=outt[:])
        psctx.close()

    return nc, dbg


MAX_LATENT_VAR = 0.1
_CACHE = {}


def _get_module(debug=False):
    key = bool(debug)
    if key not in _CACHE:
        nc, _ = build_module(debug)
        split_excess_waits(nc)
        _CACHE[key] = nc
    return _CACHE[key]


def kernel(**inputs):
    import ml_dtypes
    from concourse.bass_utils import run_bass_kernel_spmd
    nc = _get_module(False)
    x = np.asarray(inputs["x"], dtype=np.float32)
    eps = np.asarray(inputs["eps"], dtype=np.float32)
    rep = {k: np.asarray(v, dtype=np.float32) for k, v in inputs.items()
           if k not in ("x", "eps", "dec_W2", "enc_W1")}
    rep["enc_W1"] = np.ascontiguousarray(
        np.asarray(inputs["enc_W1"], dtype=np.float32)).astype(np.float16)
    w2 = np.ascontiguousarray(np.asarray(inputs["dec_W2"], dtype=np.float32))
    rep["dec_W2"] = w2.astype(ml_dtypes.float8_e4m3)
    rep["dec_W2T"] = np.ascontiguousarray(w2.T).astype(ml_dtypes.float8_e4m3)
    in_maps = []
    for c in range(NCORES):
        m = dict(rep)
        m["x"] = np.ascontiguousarray(x[c * B:(c + 1) * B])
        m["eps"] = np.ascontiguousarray(eps[0, c * B:(c + 1) * B, :])
        in_maps.append(m)
    r = run_bass_kernel_spmd(nc, in_maps, list(range(NCORES)))
    outs = np.concatenate([r.results[c]["out"] for c in range(NCORES)], axis=0)
    return (outs[:, 0], outs[:, 1], outs[:, 2], outs[:, 3], outs[:, 4])



# revision 71
# speedup vs baseline: 1.0106x; 1.0106x over previous
"""EnergyAE loss kernel for Trainium2 (Bass/Tile), 8-core data-parallel.

512-sample batch sharded 64/core; weights replicated. Returns the same
5-tuple as the reference: (neg_log_prob, recon_loss, latent_energy,
logdet_loss, sigma), each (512,) float32.

Per-core pipeline:
  S0  load x, PE-transpose to xT (D-on-partition); bias staging; bit patterns
  S1  h = tanh(x@W1+b1)  (x^T stationary, W1 streamed as k-strips)
  S2  [z*|log s] = [Wmu|Wls]^T h + bias; sigma; broadcasts; batch-layout z
  S3  decoder tanh features t,s=1-t^2,w=2ts at z*; V_aug=[diag(s)W1d^T | t]
  S4  stream W2 column-strips: J[dc]=W2[:,dc]^T V_aug (dec1 tail fused),
      packed 8-sample JTJ += J^T J, PE-transposed W2 blocks give g += W2 d^T
  S4b hess = W1d diag(2 t s g / sigma) W1d^T  (packed matmuls)
  S4c Prec_packed = JTJ*M_sigma + hess + I    (mask-matmul built operands)
  S4d unpack packed (128,(g,r,j)) -> per-sample (64, 256) via 64 tiny DMAs
  S6  Gershgorin bracket; Householder tridiagonalization; Sturm multisection
      eigmin; shift; Cholesky; U^-1; trace-inv; logdet; z_off = U^-1 eps
  S5  decoder at z_sample (W2 row-strips), recon loss, output assembly
"""
import numpy as np

import concourse.bass as bass
import concourse.tile as tile
from concourse import mybir

F32 = mybir.dt.float32
F32R = mybir.dt.float32r
BF16 = mybir.dt.bfloat16
F8E4 = mybir.dt.float8e4
F16 = mybir.dt.float16
MMPM = mybir.MatmulPerfMode
I32 = mybir.dt.int32
AX = mybir.AxisListType
ALU = mybir.AluOpType
ACTF = mybir.ActivationFunctionType
AP = bass.AP

D, H, N, BS = 3072, 2048, 16, 512
NCORES = 8
B = BS // NCORES            # 64
KC_H = H // 128             # 16
KC_D = D // 128             # 24
NGRP = B // 8               # 8
PACK = NGRP * 128           # 1024
BN = B * N                  # 1024
NSHIFT = 24                 # Sturm multisection grid
NSTURM = 2                  # multisection iterations


def _sap(t, offset, *dims):
    base = t[:]
    return AP(tensor=base.tensor, offset=base.offset + offset, ap=list(dims))


def split_excess_waits(nc, max_waits=1):
    """This walrus build accepts only one sync wait per instruction: move
    excess waits onto same-engine NoOps inserted just before."""
    n = 0
    for f in nc.m.functions:
        for bb in f.blocks:
            out = []
            for ins in bb.instructions:
                si = getattr(ins, "sync_info", None)
                ow = list(si.on_wait) if (si is not None and si.on_wait) else []
                if len(ow) > max_waits:
                    si.on_wait = ow[-max_waits:]
                    for w in ow[:-max_waits]:
                        n += 1
                        out.append(mybir.InstNoOp(
                            name=f"I-waitsplit-{n}",
                            sync_info=mybir.SyncInfo(on_wait=[w], on_update=[]),
                            bass_nofuse=True,
                            engine=ins.engine,
                        ))
                out.append(ins)
            bb.instructions = out
    return n


def build_module(debug=False):
    from contextlib import ExitStack

    nc = bass.Bass("TRN2", target_bir_lowering=False, debug=False,
                   num_devices=NCORES)

    x_d = nc.declare_dram_parameter("x", [B, D], F32R, isOutput=False)
    eps_d = nc.declare_dram_parameter("eps", [B, N], F32, isOutput=False)
    eW1_d = nc.declare_dram_parameter("enc_W1", [D, H], F16, isOutput=False)
    eb1_d = nc.declare_dram_parameter("enc_b1", [H], F32R, isOutput=False)
    eWmu_d = nc.declare_dram_parameter("enc_Wmu", [H, N], F32R, isOutput=False)
    ebmu_d = nc.declare_dram_parameter("enc_bmu", [N], F32R, isOutput=False)
    eWls_d = nc.declare_dram_parameter("enc_Wls", [H, 1], F32R, isOutput=False)
    ebls_d = nc.declare_dram_parameter("enc_bls", [1], F32R, isOutput=False)
    dW1_d = nc.declare_dram_parameter("dec_W1", [N, H], F32R, isOutput=False)
    db1_d = nc.declare_dram_parameter("dec_b1", [H], F32, isOutput=False)
    dW2_d = nc.declare_dram_parameter("dec_W2", [H, D], F8E4, isOutput=False)
    dW2T_d = nc.declare_dram_parameter("dec_W2T", [D, H], F8E4,
                                       isOutput=False)
    db2_d = nc.declare_dram_parameter("dec_b2", [D], F32R, isOutput=False)
    out_d = nc.declare_dram_parameter("out", [B, 5], F32, isOutput=True)

    dbg = {}
    if debug:
        for name, shape in [
            ("dbg_h", [B, H]), ("dbg_zsig", [B, N + 1]),
            ("dbg_t", [128, KC_H * B]), ("dbg_jtj", [128, PACK]),
            ("dbg_g", [128, KC_H * B]), ("dbg_dec1", [B, D]),
            ("dbg_hess", [128, PACK]), ("dbg_prec", [B, N * N]),
            ("dbg_tri", [B, 2 * N]), ("dbg_eig", [B, 4]),
            ("dbg_chol", [B, N * N]), ("dbg_xinv", [B, N * N]),
            ("dbg_zoff", [B, N]), ("dbg_parts", [B, 8]),
        ]:
            dbg[name] = nc.declare_dram_parameter(name, shape, F32,
                                                  isOutput=True)

    ctx = ExitStack()
    with tile.TileContext(nc) as tc, ctx:
        from contextlib import ExitStack as _ES
        per = ctx.enter_context(tc.tile_pool(name="per", bufs=1))
        dma2 = ctx.enter_context(tc.tile_pool(name="dma2", bufs=2))
        sm = ctx.enter_context(tc.tile_pool(name="sm", bufs=1))
        psctx = _ES()
        _pscur = [None]

        def psum_phase(name):
            nonlocal psctx
            psctx.close()
            psctx = _ES()
            _pscur[0] = psctx.enter_context(
                tc.tile_pool(name=name, bufs=1, space="PSUM"))
            return _pscur[0]
        V = nc.vector
        SC = nc.scalar

        def dbg_dump(name, src_ap, cast=False):
            if not debug:
                return
            nc.sync.dma_start(out=dbg[name][:],
                              in_=src_ap.bitcast(F32) if cast else src_ap)

        # ================= S0: inputs & patterns =================
        x_sb = per.tile([B, D], F32R, tag="Vbig")
        nc.sync.dma_start(out=x_sb, in_=x_d[:])
        eps_sb = per.tile([B, N], F32)
        nc.sync.dma_start(out=eps_sb, in_=eps_d[:])

        io_rowf = sm.tile([128, 128], F32)
        nc.gpsimd.iota(io_rowf[:], pattern=[[1, 128]], base=0,
                       channel_multiplier=0,
                       allow_small_or_imprecise_dtypes=True)
        pidx = sm.tile([128, 1], F32)
        nc.gpsimd.iota(pidx[:], pattern=[[0, 1]], base=0, channel_multiplier=1,
                       allow_small_or_imprecise_dtypes=True)
        ident = sm.tile([128, 128], F32R)
        V.tensor_scalar(out=ident[:], in0=io_rowf[:], scalar1=pidx[:],
                        scalar2=None, op0=ALU.is_equal)
        # bf16 identity: transposes with a bf16 moving operand cost 1.0
        # cycles/row instead of 1.5 (values are exact 0/1 in bf16)
        identb = sm.tile([128, 128], BF16)
        V.tensor_scalar(out=identb[:], in0=io_rowf[:], scalar1=pidx[:],
                        scalar2=None, op0=ALU.is_equal)
        ones_row = sm.tile([1, 128], F32R)
        V.tensor_scalar(out=ones_row[:], in0=io_rowf[0:1, :], scalar1=0.0,
                        scalar2=None, op0=ALU.is_ge)

        def pe_transpose(dst_ap, src_ap, p, f):
            pt = _pscur[0].tile([128, 128], F32R, name="pt_stage",
                                tag="pt_stage", bufs=2)
            nc.tensor.transpose(pt[:f, :p], src_ap, ident[:p, :p])
            V.tensor_copy(dst_ap, pt[:f, :p])

        psum_phase("ps0")

        xT = per.tile([128, KC_D, B], F32R)
        for dc in range(KC_D):
            pe_transpose(xT[:, dc, :], x_sb[:, dc * 128:(dc + 1) * 128], B, 128)

        db1r = sm.tile([1, H], F32R)
        nc.scalar.dma_start(out=db1r,
                            in_=AP(tensor=db1_d, offset=0,
                                   ap=[[0, 1], [1, H]]).bitcast(F32R))
        db1c = sm.tile([128, KC_H], F32)
        nc.sync.dma_start(out=db1c, in_=AP(tensor=db1_d, offset=0,
                                           ap=[[1, 128], [128, KC_H]]))

        muls = per.tile([128, KC_H, N + 1], F32R, tag="featF")
        nc.sync.dma_start(out=muls[:, :, 0:N],
                          in_=AP(tensor=eWmu_d, offset=0,
                                 ap=[[N, 128], [128 * N, KC_H], [1, N]]))
        nc.sync.dma_start(out=muls[:, :, N:N + 1],
                          in_=AP(tensor=eWls_d, offset=0,
                                 ap=[[1, 128], [128, KC_H], [0, 1]]))
        muls16 = per.tile([128, KC_H, N + 1], F16, tag="featF2")
        V.tensor_copy(muls16[:], muls[:].bitcast(F32))
        bmur = sm.tile([1, N + 1], F32R)
        nc.sync.dma_start(out=bmur[:, 0:N], in_=AP(tensor=ebmu_d, offset=0,
                                                   ap=[[0, 1], [1, N]]))
        nc.sync.dma_start(out=bmur[:, N:N + 1],
                          in_=AP(tensor=ebls_d, offset=0, ap=[[0, 1], [1, 1]]))
        # resident dec_W1: [N, kc, 128] in one DMA; transposed + bf16 copies
        w1dc_res = per.tile([N, KC_H, 128], F32R, tag="featD")
        nc.scalar.dma_start(out=w1dc_res,
                            in_=AP(tensor=dW1_d, offset=0,
                                   ap=[[H, N], [128, KC_H], [1, 128]]))
        w1dc_b16 = sm.tile([N, KC_H, 128], BF16)
        V.tensor_copy(w1dc_b16[:], w1dc_res[:].bitcast(F32))
        w1dT = per.tile([128, KC_H, N], F32R)
        for kc in range(KC_H):
            pe_transpose(w1dT[:, kc, :], w1dc_res[:, kc, :], N, 128)
        # dec_b2 in column layout [128, KC_D] (partition = d within strip)
        db2col = sm.tile([128, KC_D], F32R)
        nc.sync.dma_start(out=db2col, in_=AP(tensor=db2_d, offset=0,
                                             ap=[[1, 128], [128, KC_D]]))
        onescol_b = sm.tile([128, 1], BF16)
        V.tensor_scalar(out=onescol_b[:], in0=pidx[:], scalar1=-1.0,
                        scalar2=None, op0=ALU.is_gt)
        # resident fp8 dec_W2 as kc-pair tiles (DoubleRow lhsT needs the
        # pair dim inside one tile) and pre-transposed d-strips for g
        W2P8 = [per.tile([128, 2, D], F8E4, name=f"w2p{p}")
                for p in range(KC_H // 2)]
        W2TP8 = [per.tile([128, 2, KC_H * 128], F8E4, name=f"w2t{dp}")
                 for dp in range(KC_D // 2)]

        # ================= S1: encoder h =================
        ps = _pscur[0]
        ph = [ps.tile([B, 512], F32, name=f"ph{i}") for i in range(4)]
        for nck in range(4):
            eb1c = sm.tile([1, 512], F32R, name="eb1c", tag="b512")
            nc.scalar.dma_start(out=eb1c, in_=AP(tensor=eb1_d, offset=nck * 512,
                                               ap=[[0, 1], [1, 512]]))
            nc.tensor.matmul(ph[nck][:], ones_row[:, 0:B], eb1c[:],
                             start=True, stop=False)
        xT_f16 = per.tile([128, KC_D, B], F16)
        V.tensor_copy(xT_f16[:], xT[:].bitcast(F32))
        for kc in range(KC_D):
            for hf in range(2):
                w1s = dma2.tile([128, H // 2], F16, name="w1s",
                                tag="wstream", bufs=4)
                qeng = (nc.sync, nc.scalar, nc.gpsimd)[(2 * kc + hf) % 3]
                qeng.dma_start(
                    out=w1s, in_=eW1_d[kc * 128:(kc + 1) * 128,
                                       hf * 1024:(hf + 1) * 1024])
                for nk in range(2):
                    nck = hf * 2 + nk
                    nc.tensor.matmul(ph[nck][:], xT_f16[:, kc, :],
                                     w1s[:, nk * 512:(nk + 1) * 512],
                                     start=False, stop=(kc == KC_D - 1),
                                     skip_group_check=(kc != KC_D - 1))
        for p in range(KC_H // 2):
            (nc.sync if p % 2 == 0 else nc.scalar).dma_start(
                out=W2P8[p][:],
                in_=AP(tensor=dW2_d, offset=p * 256 * D,
                       ap=[[D, 128], [128 * D, 2], [1, D]]))
        for dp in range(KC_D // 2):
            (nc.sync if dp % 2 == 0 else nc.scalar).dma_start(
                out=W2TP8[dp][:],
                in_=AP(tensor=dW2T_d, offset=dp * 256 * H,
                       ap=[[H, 128], [128 * H, 2], [1, H]]))
        h_sb = per.tile([B, H], F32R, tag="Vbig")
        for nck in range(4):
            SC.activation(h_sb[:, nck * 512:(nck + 1) * 512], ph[nck][:],
                          ACTF.Tanh)
        dbg_dump("dbg_h", h_sb[:], cast=True)
        hT = per.tile([128, KC_H, B], F32R, tag="featD")
        hT16 = per.tile([128, KC_H, B], F16)
        for kc in range(KC_H):
            pe_transpose(hT[:, kc, :], h_sb[:, kc * 128:(kc + 1) * 128], B, 128)
        V.tensor_copy(hT16[:], hT[:].bitcast(F32))

        # ================= S2: z_star / sigma =================
        ps = psum_phase("ps2")
        pz = ps.tile([N, B], F32, name="pz")
        nc.tensor.matmul(pz[:], bmur[:, 0:N], ones_row[:, 0:B], start=True,
                         stop=False)
        for kc in range(KC_H):
            nc.tensor.matmul(pz[:], muls16[:, kc, 0:N], hT16[:, kc, :],
                             start=False, stop=(kc == KC_H - 1),
                             skip_group_check=(kc != KC_H - 1))
        pzs = ps.tile([1, B], F32, name="pzs")
        nc.tensor.matmul(pzs[:], bmur[:, N:N + 1], ones_row[:, 0:B],
                         start=True, stop=False)
        for kc in range(KC_H):
            nc.tensor.matmul(pzs[:], muls16[:, kc, N:N + 1], hT16[:, kc, :],
                             start=False, stop=(kc == KC_H - 1),
                             skip_group_check=(kc != KC_H - 1))
        zT = per.tile([N, B], F32R)
        V.tensor_copy(zT[:], pz[:])
        sig_row = sm.tile([1, B], F32R)
        SC.activation(sig_row[:], pzs[:], ACTF.Exp)
        invsigT = sm.tile([1, B], F32R)
        with nc.allow_low_precision(reason="fp32r bits are full fp32 here"):
            V.reciprocal(invsigT[:], sig_row[:].bitcast(F32))
        pb = ps.tile([128, B], F32, name="pb")
        nc.tensor.matmul(pb[:], ones_row[:, 0:128], invsigT[:],
                         start=True, stop=True)
        invsig_bc = per.tile([128, B], F32)
        V.tensor_copy(invsig_bc[:], pb[:])
        # batch layout via matmul transposes: zsig (B, 17)
        pzb = ps.tile([B, N], F32, name="pzb")
        nc.tensor.matmul(pzb[:], zT[:], ident[0:N, 0:N],
                         start=True, stop=True)
        psb = ps.tile([B, 64], F32, name="psb")
        nc.tensor.matmul(psb[:], sig_row[:], ones_row[:, 0:64],
                         start=True, stop=True)
        zsig = per.tile([B, N + 1], F32R)
        V.tensor_copy(zsig[:, 0:N], pzb[:])
        V.tensor_copy(zsig[:, N:N + 1], psb[:, 0:1])
        z_b = zsig[:, 0:N].bitcast(F32)
        sig_b = zsig[:, N:N + 1].bitcast(F32)
        dbg_dump("dbg_zsig", zsig[:], cast=True)
        invsig_b = sm.tile([B, 1], F32)
        V.reciprocal(invsig_b[:], sig_b)
        invsig2_b = sm.tile([B, 1], F32)
        V.tensor_tensor(out=invsig2_b[:], in0=invsig_b[:], in1=invsig_b[:],
                        op=ALU.mult)
        zT_b16 = sm.tile([N, B], BF16)
        V.tensor_copy(zT_b16[:], zT[:].bitcast(F32))
        neg_invsig_bc = per.tile([128, B], F32)
        V.tensor_scalar(out=neg_invsig_bc[:], in0=invsig_bc[:], scalar1=-1.0,
                        scalar2=None, op0=ALU.mult)

        # ================= S3: decoder features at z_star =================
        tT = per.tile([128, KC_H, B], BF16, tag="featB")
        sT = per.tile([128, KC_H, B], BF16, tag="featA")
        wT = per.tile([128, KC_H, B], F32, tag="featE")
        ps = psum_phase("ps3")
        for kq in range(4):
            pa = ps.tile([128, 4, B], F32, name="pa", tag="pa", bufs=2)
            for kk in range(4):
                kc = kq * 4 + kk
                nc.tensor.matmul(pa[:, kk, :],
                                 db1r[0:1, kc * 128:(kc + 1) * 128],
                                 ones_row[0:1, 0:B], start=True, stop=False)
                nc.tensor.matmul(pa[:, kk, :], w1dc_b16[:, kc, :], zT_b16[:],
                                 start=False, stop=True)
            SC.activation(tT[:, kq * 4:(kq + 1) * 4, :], pa[:], ACTF.Tanh)
            t2f = sm.tile([128, 4, B], F32, name="t2f", tag="t2f", bufs=2)
            SC.activation(t2f[:], tT[:, kq * 4:(kq + 1) * 4, :], ACTF.Square)
            V.tensor_scalar(out=sT[:, kq * 4:(kq + 1) * 4, :], in0=t2f[:],
                            scalar1=-1.0, scalar2=1.0, op0=ALU.mult,
                            op1=ALU.add)
            V.scalar_tensor_tensor(out=wT[:, kq * 4:(kq + 1) * 4, :],
                                   in0=tT[:, kq * 4:(kq + 1) * 4, :],
                                   scalar=2.0,
                                   in1=sT[:, kq * 4:(kq + 1) * 4, :],
                                   op0=ALU.mult, op1=ALU.mult)

        tT8 = per.tile([128, KC_H, B], F8E4)
        V.tensor_copy(tT8[:], tT[:])
        Vaug = per.tile([128, KC_H, BN], F8E4, tag="Vbig")
        vp = Vaug[:].ap[0][0]
        sp_ = sT[:].ap[0][0]
        wtp = w1dT[:].ap[0][0]
        for kc in range(KC_H):
            eng = V if kc < 7 else nc.gpsimd
            eng.tensor_tensor(
                out=_sap(Vaug, kc * BN, [vp, 128], [N, B], [1, N]),
                in0=_sap(sT, kc * B, [sp_, 128], [1, B], [0, N]),
                in1=_sap(w1dT, kc * N, [wtp, 128], [0, B], [1, N]).bitcast(F32),
                op=ALU.mult)

        # ===== S4: fused W2 single-pass loop =====
        # per d-strip: DMA fp32 strip -> Pool cast to resident bf16 ->
        # dec1T matmuls -> dT -> W2T transposes -> J strip -> g -> JTJ
        dT_all = per.tile([128, KC_D, B], F8E4)
        ps = psum_phase("ps4")
        Jsb2 = sm.tile([128, 2, BN], F8E4, name="Jsb2")
        pJlo = ps.tile([128, 512], F32, name="pJlo")           # 1 bank
        pJhi = ps.tile([128, 512], F32, name="pJhi")           # 1 bank
        pJTJ = ps.tile([128, NGRP, 128], F32, name="pJTJ")     # 2 banks
        pg = ps.tile([128, KC_H, B], F32, name="pgall")        # 2 banks
        for dc in range(KC_D):
            pdec = ps.tile([128, B], F32, name="pdec", tag="pdec")
            for p in range(KC_H // 2):
                nc.tensor.matmul(pdec[:],
                                 W2P8[p][:, :, dc * 128:(dc + 1) * 128],
                                 tT8[:, 2 * p:2 * p + 2, :],
                                 start=(p == 0), stop=(p == KC_H // 2 - 1),
                                 skip_group_check=(p not in
                                                   (0, KC_H // 2 - 1)),
                                 perf_mode=MMPM.DoubleRow)
            dfc = sm.tile([128, B], F32, name="dfc", tag="diff", bufs=2)
            V.scalar_tensor_tensor(out=dfc[:], in0=pdec[:],
                                   scalar=db2col[:, dc:dc + 1].bitcast(F32),
                                   in1=xT[:, dc, :].bitcast(F32),
                                   op0=ALU.add, op1=ALU.subtract)
            V.tensor_tensor(out=dT_all[:, dc, :], in0=dfc[:],
                            in1=neg_invsig_bc[:], op=ALU.mult)
            vap = Vaug[:].ap[0][0]
            NPAIR = KC_H // 2
            for pr in range(NPAIR):
                nc.tensor.matmul(
                    pJlo[:],
                    W2P8[pr][:, :, dc * 128:(dc + 1) * 128],
                    _sap(Vaug, (2 * pr) * BN, [vap, 128], [BN, 2], [1, 512]),
                    start=(pr == 0), stop=(pr == NPAIR - 1),
                    skip_group_check=(pr not in (0, NPAIR - 1)),
                    perf_mode=MMPM.DoubleRow)
            V.tensor_copy(Jsb2[:, dc % 2, 0:512], pJlo[:])
            for pr in range(NPAIR):
                nc.tensor.matmul(
                    pJhi[:],
                    W2P8[pr][:, :, dc * 128:(dc + 1) * 128],
                    _sap(Vaug, (2 * pr) * BN + 512, [vap, 128], [BN, 2],
                         [1, 512]),
                    start=(pr == 0), stop=(pr == NPAIR - 1),
                    skip_group_check=(pr not in (0, NPAIR - 1)),
                    perf_mode=MMPM.DoubleRow)
            V.tensor_copy(Jsb2[:, dc % 2, 512:1024], pJhi[:])
            if dc % 2 == 1:
                dp = dc // 2
                for kc in range(KC_H):
                    st = (dp == 0 and kc in (0, 8))
                    sp = (dp == KC_D // 2 - 1 and kc in (7, 15))
                    nc.tensor.matmul(pg[:, kc, :],
                                     W2TP8[dp][:, :,
                                               kc * 128:(kc + 1) * 128],
                                     dT_all[:, dc - 1:dc + 1, :],
                                     start=st, stop=sp,
                                     skip_group_check=not (st or sp),
                                     perf_mode=MMPM.DoubleRow)
            if dc % 2 == 1:
                dp = dc // 2
                for g in range(NGRP):
                    st = (dp == 0 and g in (0, 4))
                    sp = (dp == KC_D // 2 - 1 and g in (3, 7))
                    nc.tensor.matmul(pJTJ[:, g, :],
                                     Jsb2[:, :, g * 128:(g + 1) * 128],
                                     Jsb2[:, :, g * 128:(g + 1) * 128],
                                     start=st, stop=sp,
                                     skip_group_check=not (st or sp),
                                     perf_mode=MMPM.DoubleRow)
        JTJsb = per.tile([128, PACK], F32, tag="featD")
        V.tensor_copy(JTJsb[:], pJTJ[:].rearrange("p a b -> p (a b)"))
        gsb = per.tile([128, KC_H, B], BF16, tag="featC")
        V.tensor_tensor(out=gsb[:], in0=pg[:],
                        in1=_sap(invsig_bc, 0, [invsig_bc[:].ap[0][0], 128],
                                 [0, KC_H], [1, B]),
                        op=ALU.mult)
        dbg_dump("dbg_jtj", JTJsb[:])
        dbg_dump("dbg_g", gsb[:].rearrange("p a b -> p (a b)"))

        # ================= S4b: hess =================
        w1rep = per.tile([128, KC_H, 128], F8E4, tag="featF")
        for kc in range(KC_H):
            V.tensor_copy(w1rep[:, kc, :],
                          _sap(w1dT, kc * N, [wtp, 128], [0, 8],
                               [1, N]).bitcast(F32))
        cT = per.tile([128, KC_H, B], F32, tag="featB")
        V.tensor_tensor(out=cT[:], in0=wT[:], in1=gsb[:], op=ALU.mult)
        Vc = per.tile([128, KC_H, BN], F8E4, tag="Vbig")
        cp_ = cT[:].ap[0][0]
        for kc in range(KC_H):
            eng = V if kc < 7 else nc.gpsimd
            eng.tensor_tensor(
                out=_sap(Vc, kc * BN, [Vc[:].ap[0][0], 128], [N, B], [1, N]),
                in0=_sap(cT, kc * B, [cp_, 128], [1, B], [0, N]),
                in1=_sap(w1dT, kc * N, [wtp, 128], [0, B], [1, N]).bitcast(F32),
                op=ALU.mult)
        ps = psum_phase("ps4b")
        pH = ps.tile([128, NGRP, 128], F32, name="pH")
        for p in range(KC_H // 2):
            for g in range(NGRP):
                st = (p == 0 and g in (0, 4))
                sp = (p == KC_H // 2 - 1 and g in (3, 7))
                nc.tensor.matmul(
                    pH[:, g, :],
                    _sap(Vc, (2 * p) * BN + g * 128,
                         [Vc[:].ap[0][0], 128], [BN, 2], [1, 128]),
                    w1rep[:, 2 * p:2 * p + 2, :], start=st, stop=sp,
                    skip_group_check=not (st or sp),
                    perf_mode=MMPM.DoubleRow)
        hesssb = per.tile([128, PACK], F32, tag="featE")
        V.tensor_copy(hesssb[:], pH[:].rearrange("p a b -> p (a b)"))
        dbg_dump("dbg_hess", hesssb[:])

        # ================= S4c: Prec_packed =================
        # per-(row r, group g) scale 1/sigma^2(g*8+r) built by one mask
        # matmul; cross-sample blocks stay garbage (never read by unpack)
        ia_rf = sm.tile([B, 128], F32)
        nc.gpsimd.iota(ia_rf[:], pattern=[[1, 8], [0, 16]], base=0,
                       channel_multiplier=0,
                       allow_small_or_imprecise_dtypes=True)
        ibf = sm.tile([B, 1], F32)
        nc.gpsimd.iota(ibf[:], pattern=[[0, 1]], base=0, channel_multiplier=1,
                       allow_small_or_imprecise_dtypes=True)
        ibgf = sm.tile([B, 1], F32)
        V.memset(ibgf[:], 0.0)
        for kq in range(1, 8):
            V.scalar_tensor_tensor(out=ibgf[:], in0=ibf[:],
                                   scalar=float(8 * kq), in1=ibgf[:],
                                   op0=ALU.is_ge, op1=ALU.add)
        ib7f = sm.tile([B, 1], F32)
        V.tensor_scalar(out=ib7f[:], in0=ibgf[:], scalar1=-8.0, scalar2=None,
                        op0=ALU.mult)
        V.tensor_tensor(out=ib7f[:], in0=ibf[:], in1=ib7f[:], op=ALU.add)
        E2 = sm.tile([B, 128], F32R)
        V.tensor_scalar(out=E2[:], in0=ia_rf[:], scalar1=ib7f[:],
                        scalar2=None, op0=ALU.is_equal)
        ig8 = sm.tile([B, 8], F32)
        nc.gpsimd.iota(ig8[:], pattern=[[1, 8]], base=0, channel_multiplier=0,
                       allow_small_or_imprecise_dtypes=True)
        R2g = sm.tile([B, 8], F32R)
        V.tensor_scalar(out=R2g[:], in0=ig8[:], scalar1=ibgf[:],
                        scalar2=None, op0=ALU.is_equal)
        V.tensor_scalar(out=R2g[:], in0=R2g[:].bitcast(F32),
                        scalar1=invsig2_b[:], scalar2=None, op0=ALU.mult)
        ps2g = ps.tile([128, 8], F32, name="ps2g")
        nc.tensor.matmul(ps2g[:], E2[:], R2g[:], start=True, stop=True)
        s2g = sm.tile([128, 8], F32)
        V.tensor_copy(s2g[:], ps2g[:])
        prec_pack = JTJsb
        for g in range(NGRP):
            V.tensor_scalar(out=prec_pack[:, g * 128:(g + 1) * 128],
                            in0=prec_pack[:, g * 128:(g + 1) * 128],
                            scalar1=s2g[:, g:g + 1], scalar2=None,
                            op0=ALU.mult)
        V.tensor_tensor(out=prec_pack[:], in0=prec_pack[:], in1=hesssb[:],
                        op=ALU.add)

        # ================= S4d: unpack =================
        prec = per.tile([B, N * N], F32)
        ppp = prec_pack[:].ap[0][0]
        pp_out = prec[:].ap[0][0]
        # partition<->sample shuffle bounces through DRAM: SBUF DMAs allow
        # partition steps only in leading dims, DRAM side is unconstrained
        uscr = nc.dram_tensor("unpack_scr", [B, N * N], F32)
        for r in range(8):
            (nc.sync, nc.scalar, nc.gpsimd)[r % 3].dma_start(
                out=AP(tensor=uscr, offset=r * 256,
                       ap=[[16, 16], [8 * 256, NGRP], [1, 16]]),
                in_=_sap(prec_pack, r * 16 * ppp + r * 16,
                         [ppp, 16], [128, NGRP], [1, 16]))
        nc.sync.dma_start(out=prec[0:B // 2, :],
                          in_=AP(tensor=uscr, offset=0,
                                 ap=[[256, B // 2], [1, 256]]))
        nc.scalar.dma_start(out=prec[B // 2:B, :],
                            in_=AP(tensor=uscr, offset=(B // 2) * 256,
                                   ap=[[256, B // 2], [1, 256]]))
        dbg_dump("dbg_prec", prec[:])

        # ================= S6: eigmin =================
        pcp = prec[:].ap[0][0]

        def pdiag(t, stride=N + 1, n=N, offset=0):
            return _sap(t, offset, [t[:].ap[0][0], B], [stride, n])

        V.tensor_scalar(out=pdiag(prec), in0=pdiag(prec), scalar1=1.0,
                        scalar2=None, op0=ALU.add)
        absr = sm.tile([B, N], F32)
        V.tensor_reduce(out=absr[:],
                        in_=prec[:].rearrange("b (i j) -> b i j", i=N),
                        axis=AX.X, op=ALU.add, apply_absolute_value=True)
        dg = sm.tile([B, N], F32)
        V.tensor_copy(dg[:], pdiag(prec))
        lo_s = sm.tile([B, 1], F32)
        hi_s = sm.tile([B, 1], F32)
        lo_v = sm.tile([B, N], F32)
        V.tensor_scalar(out=lo_v[:], in0=dg[:], scalar1=2.0, scalar2=None,
                        op0=ALU.mult)
        V.tensor_tensor(out=lo_v[:], in0=lo_v[:], in1=absr[:], op=ALU.subtract)
        V.tensor_reduce(out=lo_s[:], in_=lo_v[:], axis=AX.X, op=ALU.min)
        V.tensor_reduce(out=hi_s[:], in_=dg[:], axis=AX.X, op=ALU.min)

        # --- Householder tridiagonalization ---
        A2 = per.tile([B, N * N], F32)
        V.tensor_copy(A2[:], prec[:])
        Ed = sm.tile([B, N], F32)
        V.memset(Ed[:], 0.0)
        ap2 = A2[:].ap[0][0]
        vvt = sm.tile([B, N], F32, name="vvt")
        vstep = vvt[:].ap[0][0]
        tmpm = sm.tile([B, N], F32, name="tmpm")
        qvt = sm.tile([B, N], F32, name="qvt")
        qstep = qvt[:].ap[0][0]
        omm = sm.tile([B, N * N], F32, name="omm", tag="esolv")
        s1 = sm.tile([B, 1], F32, name="s1t")
        s2 = sm.tile([B, 1], F32, name="s2t")
        s3 = sm.tile([B, 1], F32, name="s3t")
        s4 = sm.tile([B, 1], F32, name="s4t")
        for k in range(N - 2):
            m = N - 1 - k
            xap = _sap(A2, (k + 1) * N + k, [ap2, B], [N, m])
            vt = vvt[:, 0:m]
            V.tensor_copy(vt, xap)
            V.tensor_tensor(out=tmpm[:, 0:m], in0=vt, in1=vt, op=ALU.mult)
            V.tensor_reduce(out=s1[:], in_=tmpm[:, 0:m], axis=AX.X, op=ALU.add)
            SC.activation(s2[:], s1[:], ACTF.Sqrt)
            V.scalar_tensor_tensor(out=s3[:], in0=vt[:, 0:1], scalar=0.0,
                                   in1=s2[:], op0=ALU.is_ge, op1=ALU.mult)
            edk = Ed[:, k + 1:k + 2]
            V.scalar_tensor_tensor(out=edk, in0=s3[:], scalar=-2.0,
                                   in1=s2[:], op0=ALU.mult, op1=ALU.add)
            # ||v'||^2 = 2*(s1 - v0*s3) algebraically (s3^2 == s1)
            V.tensor_tensor(out=s4[:], in0=vt[:, 0:1], in1=edk, op=ALU.mult)
            V.tensor_tensor(out=s4[:], in0=s1[:], in1=s4[:], op=ALU.subtract)
            V.tensor_scalar(out=s4[:], in0=s4[:], scalar1=2.0, scalar2=1e-30,
                            op0=ALU.mult, op1=ALU.max)
            V.tensor_tensor(out=vt[:, 0:1], in0=vt[:, 0:1], in1=edk,
                            op=ALU.subtract)
            V.reciprocal(s2[:], s4[:])    # 1/||v||^2 == beta/2
            asub = _sap(A2, (k + 1) * (N + 1), [ap2, B], [N, m], [1, m])
            V.tensor_tensor(
                out=omm[:, 0:m * m].rearrange("b (i j) -> b i j", i=m),
                in0=asub,
                in1=_sap(vvt, 0, [vstep, B], [0, m], [1, m]),
                op=ALU.mult)
            pvec = tmpm[:, 0:m]
            V.tensor_reduce(out=pvec,
                            in_=omm[:, 0:m * m].rearrange("b (i j) -> b i j",
                                                          i=m),
                            axis=AX.X, op=ALU.add)
            V.tensor_tensor(out=qvt[:, 0:m], in0=pvec, in1=vt, op=ALU.mult)
            V.tensor_reduce(out=s1[:], in_=qvt[:, 0:m], axis=AX.X, op=ALU.add)
            V.tensor_tensor(out=s1[:], in0=s1[:], in1=s2[:], op=ALU.mult)
            V.tensor_scalar(out=qvt[:, 0:m], in0=vt, scalar1=s1[:],
                            scalar2=None, op0=ALU.mult)
            V.tensor_tensor(out=qvt[:, 0:m], in0=pvec, in1=qvt[:, 0:m],
                            op=ALU.subtract)
            V.tensor_scalar(out=s4[:], in0=s2[:], scalar1=-2.0, scalar2=None,
                            op0=ALU.mult)    # -beta
            V.tensor_tensor(
                out=omm[:, 0:m * m].rearrange("b (i j) -> b i j", i=m),
                in0=_sap(vvt, 0, [vstep, B], [1, m], [0, m]),
                in1=_sap(qvt, 0, [qstep, B], [0, m], [1, m]),
                op=ALU.mult)
            V.scalar_tensor_tensor(
                out=asub,
                in0=omm[:, 0:m * m].rearrange("b (i j) -> b i j", i=m),
                scalar=s4[:], in1=asub, op0=ALU.mult, op1=ALU.add)
            V.tensor_tensor(
                out=omm[:, 0:m * m].rearrange("b (i j) -> b i j", i=m),
                in0=_sap(qvt, 0, [qstep, B], [1, m], [0, m]),
                in1=_sap(vvt, 0, [vstep, B], [0, m], [1, m]),
                op=ALU.mult)
            V.scalar_tensor_tensor(
                out=asub,
                in0=omm[:, 0:m * m].rearrange("b (i j) -> b i j", i=m),
                scalar=s4[:], in1=asub, op0=ALU.mult, op1=ALU.add)
        Td = sm.tile([B, N], F32)
        V.tensor_copy(Td[:], pdiag(A2))
        nege2 = sm.tile([B, N], F32)
        V.tensor_tensor(out=nege2[:], in0=Ed[:], in1=Ed[:], op=ALU.mult)
        V.tensor_scalar(out=nege2[:], in0=nege2[:], scalar1=-1.0,
                        scalar2=-1e-30, op0=ALU.mult, op1=ALU.add)
        if debug:
            tri = sm.tile([B, 2 * N], F32, name="dbtri")
            V.tensor_copy(tri[:, 0:N], Td[:])
            V.tensor_copy(tri[:, N:2 * N], Ed[:])
            nc.sync.dma_start(out=dbg["dbg_tri"][:], in_=tri[:])

        # --- Sturm multisection ---
        iotaF = sm.tile([B, NSHIFT], F32)
        ioi2 = sm.tile([B, NSHIFT], I32)
        nc.gpsimd.iota(ioi2[:], pattern=[[1, NSHIFT]], base=1,
                       channel_multiplier=0)
        V.tensor_copy(iotaF[:], ioi2[:])
        wid = sm.tile([B, 1], F32)
        V.tensor_tensor(out=wid[:], in0=hi_s[:], in1=lo_s[:], op=ALU.subtract)
        grid = sm.tile([B, NSHIFT], F32)
        dxm = sm.tile([B, N, NSHIFT], F32, tag="scr4k_a")
        pph = sm.tile([B, NSHIFT, N], F32)
        rr = sm.tile([B, NSHIFT], F32)
        cnt = sm.tile([B, NSHIFT], F32)
        stp = sm.tile([B, 1], F32)
        for it in range(NSTURM):
            V.tensor_scalar(out=stp[:], in0=wid[:],
                            scalar1=1.0 / (NSHIFT + 1.0), scalar2=None,
                            op0=ALU.mult)
            V.tensor_scalar(out=grid[:], in0=iotaF[:], scalar1=stp[:],
                            scalar2=lo_s[:], op0=ALU.mult, op1=ALU.add)
            V.tensor_tensor(out=dxm[:],
                            in0=_sap(Td, 0, [Td[:].ap[0][0], B], [1, N],
                                     [0, NSHIFT]),
                            in1=_sap(grid, 0, [grid[:].ap[0][0], B], [0, N],
                                     [1, NSHIFT]),
                            op=ALU.subtract)
            php = pph[:].ap[0][0]
            V.tensor_copy(_sap(pph, 0, [php, B], [N, NSHIFT]), dxm[:, 0, :])
            for i in range(1, N):
                V.reciprocal(rr[:], _sap(pph, i - 1, [php, B], [N, NSHIFT]))
                V.scalar_tensor_tensor(out=_sap(pph, i, [php, B], [N, NSHIFT]),
                                       in0=rr[:],
                                       scalar=nege2[:, i:i + 1],
                                       in1=dxm[:, i, :], op0=ALU.mult,
                                       op1=ALU.add)
            V.tensor_scalar(out=pph[:], in0=pph[:], scalar1=1e-25,
                            scalar2=None, op0=ALU.is_lt)
            V.tensor_reduce(out=cnt[:], in_=pph[:],
                            axis=AX.X, op=ALU.add)
            V.tensor_scalar(out=rr[:], in0=cnt[:], scalar1=0.0, scalar2=None,
                            op0=ALU.is_equal)
            V.tensor_reduce(out=s1[:], in_=rr[:], axis=AX.X, op=ALU.add)
            V.scalar_tensor_tensor(out=lo_s[:], in0=s1[:], scalar=stp[:],
                                   in1=lo_s[:], op0=ALU.mult, op1=ALU.add)
            V.tensor_copy(wid[:], stp[:])
        eigmin = sm.tile([B, 1], F32)
        V.tensor_scalar(out=eigmin[:], in0=wid[:], scalar1=0.5,
                        scalar2=None, op0=ALU.mult)
        V.tensor_tensor(out=eigmin[:], in0=lo_s[:], in1=eigmin[:], op=ALU.add)
        delta = sm.tile([B, 1], F32)
        V.tensor_scalar(out=delta[:], in0=eigmin[:], scalar1=-1.0,
                        scalar2=10.0, op0=ALU.mult, op1=ALU.add)
        if debug:
            de = sm.tile([B, 4], F32, name="dbeig")
            V.tensor_copy(de[:, 0:1], eigmin[:])
            V.tensor_copy(de[:, 1:2], delta[:])
            V.tensor_copy(de[:, 2:3], lo_s[:])
            V.tensor_copy(de[:, 3:4], hi_s[:])
            nc.sync.dma_start(out=dbg["dbg_eig"][:], in_=de[:])

        # ============ S6b: LDL^T of Prec + delta*I (sqrt-free) ============
        U = A2  # reuse A2 storage: overwrite with a fresh copy of prec
        V.tensor_copy(U[:], prec[:])
        V.tensor_scalar(out=pdiag(U), in0=pdiag(U), scalar1=delta[:],
                        scalar2=None, op0=ALU.add)
        dvec = sm.tile([B, N], F32)   # pivots d_k
        rvec = sm.tile([B, N], F32)   # 1/d_k
        for k in range(N):
            m = N - 1 - k
            dkk = _sap(U, k * (N + 1), [ap2, B], [1, 1])
            V.reciprocal(rvec[:, k:k + 1], dkk)
            if m > 0:
                urow = _sap(U, k * N + k + 1, [ap2, B], [1, m])
                V.tensor_scalar(out=vvt[:, 0:m], in0=urow, scalar1=-1.0,
                                scalar2=None, op0=ALU.mult)   # -a
                V.tensor_scalar(out=urow, in0=urow,
                                scalar1=rvec[:, k:k + 1], scalar2=None,
                                op0=ALU.mult)                 # l = a/d
                sub = _sap(U, (k + 1) * (N + 1), [ap2, B], [N, m], [1, m])
                V.tensor_tensor(
                    out=omm[:, 0:m * m].rearrange("b (i j) -> b i j", i=m),
                    in0=_sap(vvt, 0, [vstep, B], [1, m], [0, m]),
                    in1=_sap(U, k * N + k + 1, [ap2, B], [0, m], [1, m]),
                    op=ALU.mult)                  # (-a_i) * l_j
                V.tensor_tensor(
                    out=sub, in0=sub,
                    in1=omm[:, 0:m * m].rearrange("b (i j) -> b i j", i=m),
                    op=ALU.add)
        V.tensor_copy(dvec[:], pdiag(U))
        if debug:
            dbg_dump("dbg_chol", U[:])
        # logdet_loss = 0.5 * sum log d_k
        lud = sm.tile([B, N], F32)
        logdet = sm.tile([B, 1], F32)
        SC.activation(lud[:], dvec[:], ACTF.Ln, accum_out=logdet[:])
        V.tensor_scalar(out=logdet[:], in0=logdet[:], scalar1=0.5,
                        scalar2=None, op0=ALU.mult)

        # ========= S6c: M = (L^T)^{-1} (unit diag; XT[c,j] = M[j,c]) ======
        XT = per.tile([B, N * N], F32)
        V.memset(XT[:], 0.0)
        xtp = XT[:].ap[0][0]
        for k in range(N - 1, -1, -1):
            m = N - 1 - k
            if m > 0:
                V.tensor_tensor(
                    out=omm[:, 0:N * m].rearrange("b (c j) -> b c j", c=N),
                    in0=_sap(XT, k + 1, [xtp, B], [N, N], [1, m]),
                    in1=_sap(U, k * N + k + 1, [ap2, B], [0, N], [1, m]),
                    op=ALU.mult)
                V.tensor_reduce(
                    out=tmpm[:, 0:N],
                    in_=omm[:, 0:N * m].rearrange("b (c j) -> b c j", c=N),
                    axis=AX.X, op=ALU.add)
                V.tensor_scalar(out=_sap(XT, k, [xtp, B], [N, N]),
                                in0=tmpm[:, 0:N], scalar1=-1.0,
                                scalar2=None, op0=ALU.mult)
            V.tensor_scalar(out=_sap(XT, k * N + k, [xtp, B], [1, 1]),
                            in0=_sap(XT, k * N + k, [xtp, B], [1, 1]),
                            scalar1=1.0, scalar2=None, op0=ALU.add)
        if debug:
            dbg_dump("dbg_xinv", XT[:])
        # trinv = sum_c (sum_j M[j,c]^2) / d_c ; z_off = M.T... = U^-1 eps
        msq = sm.tile([B, N * N], F32, name="xsq", tag="esolv")
        V.tensor_tensor(out=msq[:], in0=XT[:], in1=XT[:], op=ALU.mult)
        V.tensor_reduce(out=tmpm[:, 0:N],
                        in_=msq[:].rearrange("b (c j) -> b c j", c=N),
                        axis=AX.X, op=ALU.add)
        trinv = sm.tile([B, 1], F32)
        V.tensor_tensor(out=qvt[:, 0:N], in0=tmpm[:, 0:N], in1=rvec[:],
                        op=ALU.mult)
        V.tensor_reduce(out=trinv[:], in_=qvt[:, 0:N], axis=AX.X, op=ALU.add)
        sqd = sm.tile([B, N], F32)
        SC.activation(sqd[:], rvec[:], ACTF.Sqrt)   # d^{-1/2}
        eh = sm.tile([B, N], F32)
        V.tensor_tensor(out=eh[:], in0=eps_sb[:], in1=sqd[:], op=ALU.mult)
        zoffm = sm.tile([B, N, N], F32, name="zoffm", tag="esolv")
        V.tensor_tensor(out=zoffm[:],
                        in0=_sap(XT, 0, [xtp, B], [1, N], [N, N]),
                        in1=_sap(eh, 0, [eh[:].ap[0][0], B], [0, N],
                                 [1, N]),
                        op=ALU.mult)
        z_off = sm.tile([B, N], F32)
        V.tensor_reduce(out=z_off[:], in_=zoffm[:], axis=AX.X, op=ALU.add)
        dbg_dump("dbg_zoff", z_off[:])
        z_samp = per.tile([B, N], F32R)
        V.tensor_tensor(out=z_samp[:], in0=z_b, in1=z_off[:], op=ALU.add)

        # latent_energy = 0.5*(|z*|^2 + trinv)
        zsq = sm.tile([B, N], F32, name="zsq")
        zn = sm.tile([B, 1], F32)
        SC.activation(zsq[:], z_b, ACTF.Square, accum_out=zn[:])
        lat = sm.tile([B, 1], F32)
        V.tensor_tensor(out=lat[:], in0=zn[:], in1=trinv[:], op=ALU.add)
        V.tensor_scalar(out=lat[:], in0=lat[:], scalar1=0.5, scalar2=None,
                        op0=ALU.mult)

        # ================= S5: recon at z_sample (d-layout) =================
        ps = psum_phase("ps5")
        zsT = per.tile([N, B], F32R)
        pe_transpose(zsT[:], z_samp[:], B, N)
        zsT_b16 = sm.tile([N, B], BF16)
        V.tensor_copy(zsT_b16[:], zsT[:].bitcast(F32))
        t2T = per.tile([128, KC_H, B], F8E4, tag="featA")
        for kq in range(4):
            pa2 = ps.tile([128, 4, B], F32, name="pa2", tag="pa2", bufs=2)
            for kk in range(4):
                kc = kq * 4 + kk
                nc.tensor.matmul(pa2[:, kk, :],
                                 db1r[0:1, kc * 128:(kc + 1) * 128],
                                 ones_row[0:1, 0:B], start=True, stop=False)
                nc.tensor.matmul(pa2[:, kk, :], w1dc_b16[:, kc, :],
                                 zsT_b16[:], start=False, stop=True)
            SC.activation(t2T[:, kq * 4:(kq + 1) * 4, :], pa2[:], ACTF.Tanh)
        pr22 = ps.tile([B, B], F32, name="pr22")
        for dc in range(KC_D):
            prc = ps.tile([128, B], F32, name="prc", tag="prc", bufs=2)
            for p in range(KC_H // 2):
                nc.tensor.matmul(prc[:],
                                 W2P8[p][:, :, dc * 128:(dc + 1) * 128],
                                 t2T[:, 2 * p:2 * p + 2, :],
                                 start=(p == 0), stop=(p == KC_H // 2 - 1),
                                 skip_group_check=(p not in
                                                   (0, KC_H // 2 - 1)),
                                 perf_mode=MMPM.DoubleRow)
            dfr = sm.tile([128, B], BF16, name="dfr", tag="diff", bufs=2)
            V.scalar_tensor_tensor(out=dfr[:], in0=prc[:],
                                   scalar=db2col[:, dc:dc + 1].bitcast(F32),
                                   in1=xT[:, dc, :].bitcast(F32),
                                   op0=ALU.add, op1=ALU.subtract)
            nc.tensor.matmul(pr22[:], dfr[:], dfr[:],
                             start=(dc == 0), stop=(dc == KC_D - 1),
                             skip_group_check=(dc not in (0, KC_D - 1)))
        dsq = sm.tile([B, B], F32)
        V.tensor_tensor(out=dsq[:], in0=pr22[:],
                        in1=ident[0:B, 0:B].bitcast(F32), op=ALU.mult)
        r2 = sm.tile([B, 1], F32)
        V.tensor_reduce(out=r2[:], in_=dsq[:], axis=AX.X, op=ALU.add)
        recon = sm.tile([B, 1], F32)
        V.scalar_tensor_tensor(out=recon[:], in0=r2[:], scalar=0.5,
                               in1=invsig2_b[:], op0=ALU.mult, op1=ALU.mult)

        # ================= outputs =================
        lsig = sm.tile([B, 1], F32)
        SC.activation(lsig[:], sig_b, ACTF.Ln)
        nlp = sm.tile([B, 1], F32)
        V.tensor_tensor(out=nlp[:], in0=recon[:], in1=lat[:], op=ALU.add)
        V.tensor_tensor(out=nlp[:], in0=nlp[:], in1=logdet[:], op=ALU.add)
        V.tensor_scalar(out=s1[:], in0=lsig[:], scalar1=float(D), scalar2=None,
                        op0=ALU.mult)
        V.tensor_tensor(out=nlp[:], in0=nlp[:], in1=s1[:], op=ALU.add)
        V.tensor_scalar(out=nlp[:], in0=nlp[:], scalar1=1.0 / D, scalar2=None,
                        op0=ALU.mult)
        outt = sm.tile([B, 5], F32)
        V.tensor_copy(outt[:, 0:1], nlp[:])
        V.tensor_copy(outt[:, 1:2], recon[:])
        V.tensor_copy(outt[:, 2:3], lat[:])
        V.tensor_copy(outt[:, 3:4], logdet[:])
        V.tensor_copy(outt[:, 4:5], sig_b)
        nc.sync.dma_start(out=out_d[:], in_=outt[:])
        psctx.close()

    return nc, dbg


MAX_LATENT_VAR = 0.1
_CACHE = {}


def _get_module(debug=False):
    key = bool(debug)
    if key not in _CACHE:
        nc, _ = build_module(debug)
        split_excess_waits(nc)
        _CACHE[key] = nc
    return _CACHE[key]


def kernel(**inputs):
    import ml_dtypes
    from concourse.bass_utils import run_bass_kernel_spmd
    nc = _get_module(False)
    x = np.asarray(inputs["x"], dtype=np.float32)
    eps = np.asarray(inputs["eps"], dtype=np.float32)
    rep = {k: np.asarray(v, dtype=np.float32) for k, v in inputs.items()
           if k not in ("x", "eps", "dec_W2", "enc_W1")}
    rep["enc_W1"] = np.ascontiguousarray(
        np.asarray(inputs["enc_W1"], dtype=np.float32)).astype(np.float16)
    w2 = np.ascontiguousarray(np.asarray(inputs["dec_W2"], dtype=np.float32))
    rep["dec_W2"] = w2.astype(ml_dtypes.float8_e4m3)
    rep["dec_W2T"] = np.ascontiguousarray(w2.T).astype(ml_dtypes.float8_e4m3)
    in_maps = []
    for c in range(NCORES):
        m = dict(rep)
        m["x"] = np.ascontiguousarray(x[c * B:(c + 1) * B])
        m["eps"] = np.ascontiguousarray(eps[0, c * B:(c + 1) * B, :])
        in_maps.append(m)
    r = run_bass_kernel_spmd(nc, in_maps, list(range(NCORES)))
    outs = np.concatenate([r.results[c]["out"] for c in range(NCORES)], axis=0)
    return (outs[:, 0], outs[:, 1], outs[:, 2], outs[:, 3], outs[:, 4])



# revision 72
# speedup vs baseline: 1.0186x; 1.0079x over previous
"""EnergyAE loss kernel for Trainium2 (Bass/Tile), 8-core data-parallel.

512-sample batch sharded 64/core; weights replicated. Returns the same
5-tuple as the reference: (neg_log_prob, recon_loss, latent_energy,
logdet_loss, sigma), each (512,) float32.

Per-core pipeline:
  S0  load x, PE-transpose to xT (D-on-partition); bias staging; bit patterns
  S1  h = tanh(x@W1+b1)  (x^T stationary, W1 streamed as k-strips)
  S2  [z*|log s] = [Wmu|Wls]^T h + bias; sigma; broadcasts; batch-layout z
  S3  decoder tanh features t,s=1-t^2,w=2ts at z*; V_aug=[diag(s)W1d^T | t]
  S4  stream W2 column-strips: J[dc]=W2[:,dc]^T V_aug (dec1 tail fused),
      packed 8-sample JTJ += J^T J, PE-transposed W2 blocks give g += W2 d^T
  S4b hess = W1d diag(2 t s g / sigma) W1d^T  (packed matmuls)
  S4c Prec_packed = JTJ*M_sigma + hess + I    (mask-matmul built operands)
  S4d unpack packed (128,(g,r,j)) -> per-sample (64, 256) via 64 tiny DMAs
  S6  Gershgorin bracket; Householder tridiagonalization; Sturm multisection
      eigmin; shift; Cholesky; U^-1; trace-inv; logdet; z_off = U^-1 eps
  S5  decoder at z_sample (W2 row-strips), recon loss, output assembly
"""
import numpy as np

import concourse.bass as bass
import concourse.tile as tile
from concourse import mybir

F32 = mybir.dt.float32
F32R = mybir.dt.float32r
BF16 = mybir.dt.bfloat16
F8E4 = mybir.dt.float8e4
F16 = mybir.dt.float16
MMPM = mybir.MatmulPerfMode
I32 = mybir.dt.int32
AX = mybir.AxisListType
ALU = mybir.AluOpType
ACTF = mybir.ActivationFunctionType
AP = bass.AP

D, H, N, BS = 3072, 2048, 16, 512
NCORES = 8
B = BS // NCORES            # 64
KC_H = H // 128             # 16
KC_D = D // 128             # 24
NGRP = B // 8               # 8
PACK = NGRP * 128           # 1024
BN = B * N                  # 1024
NSHIFT = 24                 # Sturm multisection grid
NSTURM = 2                  # multisection iterations


def _sap(t, offset, *dims):
    base = t[:]
    return AP(tensor=base.tensor, offset=base.offset + offset, ap=list(dims))


def split_excess_waits(nc, max_waits=1):
    """This walrus build accepts only one sync wait per instruction: move
    excess waits onto same-engine NoOps inserted just before."""
    n = 0
    for f in nc.m.functions:
        for bb in f.blocks:
            out = []
            for ins in bb.instructions:
                si = getattr(ins, "sync_info", None)
                ow = list(si.on_wait) if (si is not None and si.on_wait) else []
                if len(ow) > max_waits:
                    si.on_wait = ow[-max_waits:]
                    for w in ow[:-max_waits]:
                        n += 1
                        out.append(mybir.InstNoOp(
                            name=f"I-waitsplit-{n}",
                            sync_info=mybir.SyncInfo(on_wait=[w], on_update=[]),
                            bass_nofuse=True,
                            engine=ins.engine,
                        ))
                out.append(ins)
            bb.instructions = out
    return n


def build_module(debug=False):
    from contextlib import ExitStack

    nc = bass.Bass("TRN2", target_bir_lowering=False, debug=False,
                   num_devices=NCORES)

    x_d = nc.declare_dram_parameter("x", [B, D], F32R, isOutput=False)
    eps_d = nc.declare_dram_parameter("eps", [B, N], F32, isOutput=False)
    eW1_d = nc.declare_dram_parameter("enc_W1", [D, H], F16, isOutput=False)
    eb1_d = nc.declare_dram_parameter("enc_b1", [H], F32R, isOutput=False)
    eWmu_d = nc.declare_dram_parameter("enc_Wmu", [H, N], F32R, isOutput=False)
    ebmu_d = nc.declare_dram_parameter("enc_bmu", [N], F32R, isOutput=False)
    eWls_d = nc.declare_dram_parameter("enc_Wls", [H, 1], F32R, isOutput=False)
    ebls_d = nc.declare_dram_parameter("enc_bls", [1], F32R, isOutput=False)
    dW1_d = nc.declare_dram_parameter("dec_W1", [N, H], F32R, isOutput=False)
    db1_d = nc.declare_dram_parameter("dec_b1", [H], F32, isOutput=False)
    dW2_d = nc.declare_dram_parameter("dec_W2", [H, D], F8E4, isOutput=False)
    dW2T_d = nc.declare_dram_parameter("dec_W2T", [D, H], F8E4,
                                       isOutput=False)
    db2_d = nc.declare_dram_parameter("dec_b2", [D], F32R, isOutput=False)
    out_d = nc.declare_dram_parameter("out", [B, 5], F32, isOutput=True)

    dbg = {}
    if debug:
        for name, shape in [
            ("dbg_h", [B, H]), ("dbg_zsig", [B, N + 1]),
            ("dbg_t", [128, KC_H * B]), ("dbg_jtj", [128, PACK]),
            ("dbg_g", [128, KC_H * B]), ("dbg_dec1", [B, D]),
            ("dbg_hess", [128, PACK]), ("dbg_prec", [B, N * N]),
            ("dbg_tri", [B, 2 * N]), ("dbg_eig", [B, 4]),
            ("dbg_chol", [B, N * N]), ("dbg_xinv", [B, N * N]),
            ("dbg_zoff", [B, N]), ("dbg_parts", [B, 8]),
        ]:
            dbg[name] = nc.declare_dram_parameter(name, shape, F32,
                                                  isOutput=True)

    ctx = ExitStack()
    with tile.TileContext(nc) as tc, ctx:
        from contextlib import ExitStack as _ES
        per = ctx.enter_context(tc.tile_pool(name="per", bufs=1))
        dma2 = ctx.enter_context(tc.tile_pool(name="dma2", bufs=2))
        sm = ctx.enter_context(tc.tile_pool(name="sm", bufs=1))
        psctx = _ES()
        _pscur = [None]

        def psum_phase(name):
            nonlocal psctx
            psctx.close()
            psctx = _ES()
            _pscur[0] = psctx.enter_context(
                tc.tile_pool(name=name, bufs=1, space="PSUM"))
            return _pscur[0]
        V = nc.vector
        SC = nc.scalar

        def dbg_dump(name, src_ap, cast=False):
            if not debug:
                return
            nc.sync.dma_start(out=dbg[name][:],
                              in_=src_ap.bitcast(F32) if cast else src_ap)

        # ================= S0: inputs & patterns =================
        x_sb = per.tile([B, D], F32R, tag="Vbig")
        nc.sync.dma_start(out=x_sb, in_=x_d[:])
        eps_sb = per.tile([B, N], F32)
        nc.sync.dma_start(out=eps_sb, in_=eps_d[:])

        io_rowf = sm.tile([128, 128], F32)
        nc.gpsimd.iota(io_rowf[:], pattern=[[1, 128]], base=0,
                       channel_multiplier=0,
                       allow_small_or_imprecise_dtypes=True)
        pidx = sm.tile([128, 1], F32)
        nc.gpsimd.iota(pidx[:], pattern=[[0, 1]], base=0, channel_multiplier=1,
                       allow_small_or_imprecise_dtypes=True)
        ident = sm.tile([128, 128], F32R)
        V.tensor_scalar(out=ident[:], in0=io_rowf[:], scalar1=pidx[:],
                        scalar2=None, op0=ALU.is_equal)
        # bf16 identity: transposes with a bf16 moving operand cost 1.0
        # cycles/row instead of 1.5 (values are exact 0/1 in bf16)
        identb = sm.tile([128, 128], BF16)
        V.tensor_scalar(out=identb[:], in0=io_rowf[:], scalar1=pidx[:],
                        scalar2=None, op0=ALU.is_equal)
        ones_row = sm.tile([1, 128], F32R)
        V.tensor_scalar(out=ones_row[:], in0=io_rowf[0:1, :], scalar1=0.0,
                        scalar2=None, op0=ALU.is_ge)

        def pe_transpose(dst_ap, src_ap, p, f):
            pt = _pscur[0].tile([128, 128], F32R, name="pt_stage",
                                tag="pt_stage", bufs=2)
            nc.tensor.transpose(pt[:f, :p], src_ap, ident[:p, :p])
            V.tensor_copy(dst_ap, pt[:f, :p])

        psum_phase("ps0")

        xT = per.tile([128, KC_D, B], F32R)
        for dc in range(KC_D):
            pe_transpose(xT[:, dc, :], x_sb[:, dc * 128:(dc + 1) * 128], B, 128)

        db1r = sm.tile([1, H], F32R)
        nc.scalar.dma_start(out=db1r,
                            in_=AP(tensor=db1_d, offset=0,
                                   ap=[[0, 1], [1, H]]).bitcast(F32R))
        db1c = sm.tile([128, KC_H], F32)
        nc.sync.dma_start(out=db1c, in_=AP(tensor=db1_d, offset=0,
                                           ap=[[1, 128], [128, KC_H]]))

        muls = per.tile([128, KC_H, N + 1], F32R, tag="featF")
        nc.sync.dma_start(out=muls[:, :, 0:N],
                          in_=AP(tensor=eWmu_d, offset=0,
                                 ap=[[N, 128], [128 * N, KC_H], [1, N]]))
        nc.sync.dma_start(out=muls[:, :, N:N + 1],
                          in_=AP(tensor=eWls_d, offset=0,
                                 ap=[[1, 128], [128, KC_H], [0, 1]]))
        muls16 = per.tile([128, KC_H, N + 1], F16, tag="featF2")
        V.tensor_copy(muls16[:], muls[:].bitcast(F32))
        bmur = sm.tile([1, N + 1], F32R)
        nc.sync.dma_start(out=bmur[:, 0:N], in_=AP(tensor=ebmu_d, offset=0,
                                                   ap=[[0, 1], [1, N]]))
        nc.sync.dma_start(out=bmur[:, N:N + 1],
                          in_=AP(tensor=ebls_d, offset=0, ap=[[0, 1], [1, 1]]))
        # resident dec_W1: [N, kc, 128] in one DMA; transposed + bf16 copies
        w1dc_res = per.tile([N, KC_H, 128], F32R, tag="featD")
        nc.scalar.dma_start(out=w1dc_res,
                            in_=AP(tensor=dW1_d, offset=0,
                                   ap=[[H, N], [128, KC_H], [1, 128]]))
        w1dc_b16 = sm.tile([N, KC_H, 128], BF16)
        V.tensor_copy(w1dc_b16[:], w1dc_res[:].bitcast(F32))
        w1dT = per.tile([128, KC_H, N], F32R)
        for kc in range(KC_H):
            pe_transpose(w1dT[:, kc, :], w1dc_res[:, kc, :], N, 128)
        # dec_b2 in column layout [128, KC_D] (partition = d within strip)
        db2col = sm.tile([128, KC_D], F32R)
        nc.sync.dma_start(out=db2col, in_=AP(tensor=db2_d, offset=0,
                                             ap=[[1, 128], [128, KC_D]]))
        onescol_b = sm.tile([128, 1], BF16)
        V.tensor_scalar(out=onescol_b[:], in0=pidx[:], scalar1=-1.0,
                        scalar2=None, op0=ALU.is_gt)
        # resident fp8 dec_W2 as kc-pair tiles (DoubleRow lhsT needs the
        # pair dim inside one tile) and pre-transposed d-strips for g
        W2P8 = [per.tile([128, 2, D], F8E4, name=f"w2p{p}")
                for p in range(KC_H // 2)]
        W2TP8 = [per.tile([128, 2, KC_H * 128], F8E4, name=f"w2t{dp}")
                 for dp in range(KC_D // 2)]

        # ================= S1: encoder h =================
        ps = _pscur[0]
        ph = [ps.tile([B, 512], F32, name=f"ph{i}") for i in range(4)]
        for nck in range(4):
            eb1c = sm.tile([1, 512], F32R, name="eb1c", tag="b512")
            nc.scalar.dma_start(out=eb1c, in_=AP(tensor=eb1_d, offset=nck * 512,
                                               ap=[[0, 1], [1, 512]]))
            nc.tensor.matmul(ph[nck][:], ones_row[:, 0:B], eb1c[:],
                             start=True, stop=False)
        xT_f16 = per.tile([128, KC_D, B], F16)
        V.tensor_copy(xT_f16[:], xT[:].bitcast(F32))
        for kc in range(KC_D):
            for hf in range(2):
                w1s = dma2.tile([128, H // 2], F16, name="w1s",
                                tag="wstream", bufs=4)
                qeng = (nc.sync, nc.scalar, nc.gpsimd)[(2 * kc + hf) % 3]
                qeng.dma_start(
                    out=w1s, in_=eW1_d[kc * 128:(kc + 1) * 128,
                                       hf * 1024:(hf + 1) * 1024])
                for nk in range(2):
                    nck = hf * 2 + nk
                    nc.tensor.matmul(ph[nck][:], xT_f16[:, kc, :],
                                     w1s[:, nk * 512:(nk + 1) * 512],
                                     start=False, stop=(kc == KC_D - 1),
                                     skip_group_check=(kc != KC_D - 1))
        for p in range(KC_H // 2):
            (nc.sync if p % 2 == 0 else nc.scalar).dma_start(
                out=W2P8[p][:],
                in_=AP(tensor=dW2_d, offset=p * 256 * D,
                       ap=[[D, 128], [128 * D, 2], [1, D]]))
        for dp in range(KC_D // 2):
            (nc.sync, nc.scalar, nc.gpsimd)[dp % 3].dma_start(
                out=W2TP8[dp][:],
                in_=AP(tensor=dW2T_d, offset=dp * 256 * H,
                       ap=[[H, 128], [128 * H, 2], [1, H]]))
        h_sb = per.tile([B, H], F32R, tag="Vbig")
        for nck in range(4):
            SC.activation(h_sb[:, nck * 512:(nck + 1) * 512], ph[nck][:],
                          ACTF.Tanh)
        dbg_dump("dbg_h", h_sb[:], cast=True)
        hT = per.tile([128, KC_H, B], F32R, tag="featD")
        hT16 = per.tile([128, KC_H, B], F16)
        for kc in range(KC_H):
            pe_transpose(hT[:, kc, :], h_sb[:, kc * 128:(kc + 1) * 128], B, 128)
        V.tensor_copy(hT16[:], hT[:].bitcast(F32))

        # ================= S2: z_star / sigma =================
        ps = psum_phase("ps2")
        pz = ps.tile([N, B], F32, name="pz")
        nc.tensor.matmul(pz[:], bmur[:, 0:N], ones_row[:, 0:B], start=True,
                         stop=False)
        for kc in range(KC_H):
            nc.tensor.matmul(pz[:], muls16[:, kc, 0:N], hT16[:, kc, :],
                             start=False, stop=(kc == KC_H - 1),
                             skip_group_check=(kc != KC_H - 1))
        pzs = ps.tile([1, B], F32, name="pzs")
        nc.tensor.matmul(pzs[:], bmur[:, N:N + 1], ones_row[:, 0:B],
                         start=True, stop=False)
        for kc in range(KC_H):
            nc.tensor.matmul(pzs[:], muls16[:, kc, N:N + 1], hT16[:, kc, :],
                             start=False, stop=(kc == KC_H - 1),
                             skip_group_check=(kc != KC_H - 1))
        zT = per.tile([N, B], F32R)
        V.tensor_copy(zT[:], pz[:])
        sig_row = sm.tile([1, B], F32R)
        SC.activation(sig_row[:], pzs[:], ACTF.Exp)
        invsigT = sm.tile([1, B], F32R)
        with nc.allow_low_precision(reason="fp32r bits are full fp32 here"):
            V.reciprocal(invsigT[:], sig_row[:].bitcast(F32))
        pb = ps.tile([128, B], F32, name="pb")
        nc.tensor.matmul(pb[:], ones_row[:, 0:128], invsigT[:],
                         start=True, stop=True)
        invsig_bc = per.tile([128, B], F32)
        V.tensor_copy(invsig_bc[:], pb[:])
        # batch layout via matmul transposes: zsig (B, 17)
        pzb = ps.tile([B, N], F32, name="pzb")
        nc.tensor.matmul(pzb[:], zT[:], ident[0:N, 0:N],
                         start=True, stop=True)
        psb = ps.tile([B, 64], F32, name="psb")
        nc.tensor.matmul(psb[:], sig_row[:], ones_row[:, 0:64],
                         start=True, stop=True)
        zsig = per.tile([B, N + 1], F32R)
        V.tensor_copy(zsig[:, 0:N], pzb[:])
        V.tensor_copy(zsig[:, N:N + 1], psb[:, 0:1])
        z_b = zsig[:, 0:N].bitcast(F32)
        sig_b = zsig[:, N:N + 1].bitcast(F32)
        dbg_dump("dbg_zsig", zsig[:], cast=True)
        invsig_b = sm.tile([B, 1], F32)
        V.reciprocal(invsig_b[:], sig_b)
        invsig2_b = sm.tile([B, 1], F32)
        V.tensor_tensor(out=invsig2_b[:], in0=invsig_b[:], in1=invsig_b[:],
                        op=ALU.mult)
        lsig = sm.tile([B, 1], F32)
        SC.activation(lsig[:], sig_b, ACTF.Ln)
        zT_b16 = sm.tile([N, B], BF16)
        V.tensor_copy(zT_b16[:], zT[:].bitcast(F32))
        neg_invsig_bc = per.tile([128, B], F32)
        V.tensor_scalar(out=neg_invsig_bc[:], in0=invsig_bc[:], scalar1=-1.0,
                        scalar2=None, op0=ALU.mult)

        # ================= S3: decoder features at z_star =================
        tT = per.tile([128, KC_H, B], BF16, tag="featB")
        sT = per.tile([128, KC_H, B], BF16, tag="featA")
        wT = per.tile([128, KC_H, B], F32, tag="featE")
        ps = psum_phase("ps3")
        for kq in range(4):
            pa = ps.tile([128, 4, B], F32, name="pa", tag="pa", bufs=2)
            for kk in range(4):
                kc = kq * 4 + kk
                nc.tensor.matmul(pa[:, kk, :],
                                 db1r[0:1, kc * 128:(kc + 1) * 128],
                                 ones_row[0:1, 0:B], start=True, stop=False)
                nc.tensor.matmul(pa[:, kk, :], w1dc_b16[:, kc, :], zT_b16[:],
                                 start=False, stop=True)
            SC.activation(tT[:, kq * 4:(kq + 1) * 4, :], pa[:], ACTF.Tanh)
            t2f = sm.tile([128, 4, B], F32, name="t2f", tag="t2f", bufs=2)
            SC.activation(t2f[:], tT[:, kq * 4:(kq + 1) * 4, :], ACTF.Square)
            V.tensor_scalar(out=sT[:, kq * 4:(kq + 1) * 4, :], in0=t2f[:],
                            scalar1=-1.0, scalar2=1.0, op0=ALU.mult,
                            op1=ALU.add)
            V.scalar_tensor_tensor(out=wT[:, kq * 4:(kq + 1) * 4, :],
                                   in0=tT[:, kq * 4:(kq + 1) * 4, :],
                                   scalar=2.0,
                                   in1=sT[:, kq * 4:(kq + 1) * 4, :],
                                   op0=ALU.mult, op1=ALU.mult)

        tT8 = per.tile([128, KC_H, B], F8E4)
        V.tensor_copy(tT8[:], tT[:])
        Vaug = per.tile([128, KC_H, BN], F8E4, tag="Vbig")
        vp = Vaug[:].ap[0][0]
        sp_ = sT[:].ap[0][0]
        wtp = w1dT[:].ap[0][0]
        for kc in range(KC_H):
            eng = V if kc < 7 else nc.gpsimd
            eng.tensor_tensor(
                out=_sap(Vaug, kc * BN, [vp, 128], [N, B], [1, N]),
                in0=_sap(sT, kc * B, [sp_, 128], [1, B], [0, N]),
                in1=_sap(w1dT, kc * N, [wtp, 128], [0, B], [1, N]).bitcast(F32),
                op=ALU.mult)

        # ===== S4: fused W2 single-pass loop =====
        # per d-strip: DMA fp32 strip -> Pool cast to resident bf16 ->
        # dec1T matmuls -> dT -> W2T transposes -> J strip -> g -> JTJ
        dT_all = per.tile([128, KC_D, B], F8E4)
        ps = psum_phase("ps4")
        Jsb2 = sm.tile([128, 2, BN], F8E4, name="Jsb2")
        pJlo = ps.tile([128, 512], F32, name="pJlo")           # 1 bank
        pJhi = ps.tile([128, 512], F32, name="pJhi")           # 1 bank
        pJTJ = ps.tile([128, NGRP, 128], F32, name="pJTJ")     # 2 banks
        pg = ps.tile([128, KC_H, B], F32, name="pgall")        # 2 banks
        for dc in range(KC_D):
            pdec = ps.tile([128, B], F32, name="pdec", tag="pdec")
            for p in range(KC_H // 2):
                nc.tensor.matmul(pdec[:],
                                 W2P8[p][:, :, dc * 128:(dc + 1) * 128],
                                 tT8[:, 2 * p:2 * p + 2, :],
                                 start=(p == 0), stop=(p == KC_H // 2 - 1),
                                 skip_group_check=(p not in
                                                   (0, KC_H // 2 - 1)),
                                 perf_mode=MMPM.DoubleRow)
            dfc = sm.tile([128, B], F32, name="dfc", tag="diff", bufs=2)
            V.scalar_tensor_tensor(out=dfc[:], in0=pdec[:],
                                   scalar=db2col[:, dc:dc + 1].bitcast(F32),
                                   in1=xT[:, dc, :].bitcast(F32),
                                   op0=ALU.add, op1=ALU.subtract)
            V.tensor_tensor(out=dT_all[:, dc, :], in0=dfc[:],
                            in1=neg_invsig_bc[:], op=ALU.mult)
            vap = Vaug[:].ap[0][0]
            NPAIR = KC_H // 2
            for pr in range(NPAIR):
                nc.tensor.matmul(
                    pJlo[:],
                    W2P8[pr][:, :, dc * 128:(dc + 1) * 128],
                    _sap(Vaug, (2 * pr) * BN, [vap, 128], [BN, 2], [1, 512]),
                    start=(pr == 0), stop=(pr == NPAIR - 1),
                    skip_group_check=(pr not in (0, NPAIR - 1)),
                    perf_mode=MMPM.DoubleRow)
            V.tensor_copy(Jsb2[:, dc % 2, 0:512], pJlo[:])
            for pr in range(NPAIR):
                nc.tensor.matmul(
                    pJhi[:],
                    W2P8[pr][:, :, dc * 128:(dc + 1) * 128],
                    _sap(Vaug, (2 * pr) * BN + 512, [vap, 128], [BN, 2],
                         [1, 512]),
                    start=(pr == 0), stop=(pr == NPAIR - 1),
                    skip_group_check=(pr not in (0, NPAIR - 1)),
                    perf_mode=MMPM.DoubleRow)
            V.tensor_copy(Jsb2[:, dc % 2, 512:1024], pJhi[:])
            if dc % 2 == 1:
                dp = dc // 2
                for kc in range(KC_H):
                    st = (dp == 0 and kc in (0, 8))
                    sp = (dp == KC_D // 2 - 1 and kc in (7, 15))
                    nc.tensor.matmul(pg[:, kc, :],
                                     W2TP8[dp][:, :,
                                               kc * 128:(kc + 1) * 128],
                                     dT_all[:, dc - 1:dc + 1, :],
                                     start=st, stop=sp,
                                     skip_group_check=not (st or sp),
                                     perf_mode=MMPM.DoubleRow)
            if dc % 2 == 1:
                dp = dc // 2
                for g in range(NGRP):
                    st = (dp == 0 and g in (0, 4))
                    sp = (dp == KC_D // 2 - 1 and g in (3, 7))
                    nc.tensor.matmul(pJTJ[:, g, :],
                                     Jsb2[:, :, g * 128:(g + 1) * 128],
                                     Jsb2[:, :, g * 128:(g + 1) * 128],
                                     start=st, stop=sp,
                                     skip_group_check=not (st or sp),
                                     perf_mode=MMPM.DoubleRow)
        JTJsb = per.tile([128, PACK], F32, tag="featD")
        V.tensor_copy(JTJsb[:], pJTJ[:].rearrange("p a b -> p (a b)"))
        gsb = per.tile([128, KC_H, B], BF16, tag="featC")
        V.tensor_tensor(out=gsb[:], in0=pg[:],
                        in1=_sap(invsig_bc, 0, [invsig_bc[:].ap[0][0], 128],
                                 [0, KC_H], [1, B]),
                        op=ALU.mult)
        dbg_dump("dbg_jtj", JTJsb[:])
        dbg_dump("dbg_g", gsb[:].rearrange("p a b -> p (a b)"))

        # ================= S4b: hess =================
        w1rep = per.tile([128, KC_H, 128], F8E4, tag="featF")
        for kc in range(KC_H):
            V.tensor_copy(w1rep[:, kc, :],
                          _sap(w1dT, kc * N, [wtp, 128], [0, 8],
                               [1, N]).bitcast(F32))
        cT = per.tile([128, KC_H, B], F32, tag="featB")
        V.tensor_tensor(out=cT[:], in0=wT[:], in1=gsb[:], op=ALU.mult)
        Vc = per.tile([128, KC_H, BN], F8E4, tag="Vbig")
        cp_ = cT[:].ap[0][0]
        for kc in range(KC_H):
            eng = V if kc < 7 else nc.gpsimd
            eng.tensor_tensor(
                out=_sap(Vc, kc * BN, [Vc[:].ap[0][0], 128], [N, B], [1, N]),
                in0=_sap(cT, kc * B, [cp_, 128], [1, B], [0, N]),
                in1=_sap(w1dT, kc * N, [wtp, 128], [0, B], [1, N]).bitcast(F32),
                op=ALU.mult)
        ps = psum_phase("ps4b")
        pH = ps.tile([128, NGRP, 128], F32, name="pH")
        for p in range(KC_H // 2):
            for g in range(NGRP):
                st = (p == 0 and g in (0, 4))
                sp = (p == KC_H // 2 - 1 and g in (3, 7))
                nc.tensor.matmul(
                    pH[:, g, :],
                    _sap(Vc, (2 * p) * BN + g * 128,
                         [Vc[:].ap[0][0], 128], [BN, 2], [1, 128]),
                    w1rep[:, 2 * p:2 * p + 2, :], start=st, stop=sp,
                    skip_group_check=not (st or sp),
                    perf_mode=MMPM.DoubleRow)
        hesssb = per.tile([128, PACK], F32, tag="featE")
        V.tensor_copy(hesssb[:], pH[:].rearrange("p a b -> p (a b)"))
        dbg_dump("dbg_hess", hesssb[:])

        # ================= S4c: Prec_packed =================
        # per-(row r, group g) scale 1/sigma^2(g*8+r) built by one mask
        # matmul; cross-sample blocks stay garbage (never read by unpack)
        ia_rf = sm.tile([B, 128], F32)
        nc.gpsimd.iota(ia_rf[:], pattern=[[1, 8], [0, 16]], base=0,
                       channel_multiplier=0,
                       allow_small_or_imprecise_dtypes=True)
        ibf = sm.tile([B, 1], F32)
        nc.gpsimd.iota(ibf[:], pattern=[[0, 1]], base=0, channel_multiplier=1,
                       allow_small_or_imprecise_dtypes=True)
        ibgf = sm.tile([B, 1], F32)
        V.memset(ibgf[:], 0.0)
        for kq in range(1, 8):
            V.scalar_tensor_tensor(out=ibgf[:], in0=ibf[:],
                                   scalar=float(8 * kq), in1=ibgf[:],
                                   op0=ALU.is_ge, op1=ALU.add)
        ib7f = sm.tile([B, 1], F32)
        V.tensor_scalar(out=ib7f[:], in0=ibgf[:], scalar1=-8.0, scalar2=None,
                        op0=ALU.mult)
        V.tensor_tensor(out=ib7f[:], in0=ibf[:], in1=ib7f[:], op=ALU.add)
        E2 = sm.tile([B, 128], F32R)
        V.tensor_scalar(out=E2[:], in0=ia_rf[:], scalar1=ib7f[:],
                        scalar2=None, op0=ALU.is_equal)
        ig8 = sm.tile([B, 8], F32)
        nc.gpsimd.iota(ig8[:], pattern=[[1, 8]], base=0, channel_multiplier=0,
                       allow_small_or_imprecise_dtypes=True)
        R2g = sm.tile([B, 8], F32R)
        V.tensor_scalar(out=R2g[:], in0=ig8[:], scalar1=ibgf[:],
                        scalar2=None, op0=ALU.is_equal)
        V.tensor_scalar(out=R2g[:], in0=R2g[:].bitcast(F32),
                        scalar1=invsig2_b[:], scalar2=None, op0=ALU.mult)
        ps2g = ps.tile([128, 8], F32, name="ps2g")
        nc.tensor.matmul(ps2g[:], E2[:], R2g[:], start=True, stop=True)
        s2g = sm.tile([128, 8], F32)
        V.tensor_copy(s2g[:], ps2g[:])
        prec_pack = JTJsb
        for g in range(NGRP):
            V.tensor_scalar(out=prec_pack[:, g * 128:(g + 1) * 128],
                            in0=prec_pack[:, g * 128:(g + 1) * 128],
                            scalar1=s2g[:, g:g + 1], scalar2=None,
                            op0=ALU.mult)
        V.tensor_tensor(out=prec_pack[:], in0=prec_pack[:], in1=hesssb[:],
                        op=ALU.add)

        # ================= S4d: unpack =================
        prec = per.tile([B, N * N], F32)
        ppp = prec_pack[:].ap[0][0]
        pp_out = prec[:].ap[0][0]
        # partition<->sample shuffle bounces through DRAM: SBUF DMAs allow
        # partition steps only in leading dims, DRAM side is unconstrained
        uscr = nc.dram_tensor("unpack_scr", [B, N * N], F32)
        for r in range(8):
            (nc.sync, nc.scalar, nc.gpsimd)[r % 3].dma_start(
                out=AP(tensor=uscr, offset=r * 256,
                       ap=[[16, 16], [8 * 256, NGRP], [1, 16]]),
                in_=_sap(prec_pack, r * 16 * ppp + r * 16,
                         [ppp, 16], [128, NGRP], [1, 16]))
        nc.sync.dma_start(out=prec[0:B // 2, :],
                          in_=AP(tensor=uscr, offset=0,
                                 ap=[[256, B // 2], [1, 256]]))
        nc.scalar.dma_start(out=prec[B // 2:B, :],
                            in_=AP(tensor=uscr, offset=(B // 2) * 256,
                                   ap=[[256, B // 2], [1, 256]]))
        dbg_dump("dbg_prec", prec[:])

        # ================= S6: eigmin =================
        pcp = prec[:].ap[0][0]

        def pdiag(t, stride=N + 1, n=N, offset=0):
            return _sap(t, offset, [t[:].ap[0][0], B], [stride, n])

        V.tensor_scalar(out=pdiag(prec), in0=pdiag(prec), scalar1=1.0,
                        scalar2=None, op0=ALU.add)
        absr = sm.tile([B, N], F32)
        V.tensor_reduce(out=absr[:],
                        in_=prec[:].rearrange("b (i j) -> b i j", i=N),
                        axis=AX.X, op=ALU.add, apply_absolute_value=True)
        dg = sm.tile([B, N], F32)
        V.tensor_copy(dg[:], pdiag(prec))
        lo_s = sm.tile([B, 1], F32)
        hi_s = sm.tile([B, 1], F32)
        lo_v = sm.tile([B, N], F32)
        V.tensor_scalar(out=lo_v[:], in0=dg[:], scalar1=2.0, scalar2=None,
                        op0=ALU.mult)
        V.tensor_tensor(out=lo_v[:], in0=lo_v[:], in1=absr[:], op=ALU.subtract)
        V.tensor_reduce(out=lo_s[:], in_=lo_v[:], axis=AX.X, op=ALU.min)
        V.tensor_reduce(out=hi_s[:], in_=dg[:], axis=AX.X, op=ALU.min)

        # --- Householder tridiagonalization ---
        A2 = per.tile([B, N * N], F32)
        V.tensor_copy(A2[:], prec[:])
        Ed = sm.tile([B, N], F32)
        V.memset(Ed[:], 0.0)
        ap2 = A2[:].ap[0][0]
        vvt = sm.tile([B, N], F32, name="vvt")
        vstep = vvt[:].ap[0][0]
        tmpm = sm.tile([B, N], F32, name="tmpm")
        qvt = sm.tile([B, N], F32, name="qvt")
        qstep = qvt[:].ap[0][0]
        omm = sm.tile([B, N * N], F32, name="omm", tag="esolv")
        s1 = sm.tile([B, 1], F32, name="s1t")
        s2 = sm.tile([B, 1], F32, name="s2t")
        s3 = sm.tile([B, 1], F32, name="s3t")
        s4 = sm.tile([B, 1], F32, name="s4t")
        for k in range(N - 2):
            m = N - 1 - k
            xap = _sap(A2, (k + 1) * N + k, [ap2, B], [N, m])
            vt = vvt[:, 0:m]
            V.tensor_copy(vt, xap)
            V.tensor_tensor(out=tmpm[:, 0:m], in0=vt, in1=vt, op=ALU.mult)
            V.tensor_reduce(out=s1[:], in_=tmpm[:, 0:m], axis=AX.X, op=ALU.add)
            SC.activation(s2[:], s1[:], ACTF.Sqrt)
            V.scalar_tensor_tensor(out=s3[:], in0=vt[:, 0:1], scalar=0.0,
                                   in1=s2[:], op0=ALU.is_ge, op1=ALU.mult)
            edk = Ed[:, k + 1:k + 2]
            V.scalar_tensor_tensor(out=edk, in0=s3[:], scalar=-2.0,
                                   in1=s2[:], op0=ALU.mult, op1=ALU.add)
            # ||v'||^2 = 2*(s1 - v0*s3) algebraically (s3^2 == s1)
            V.tensor_tensor(out=s4[:], in0=vt[:, 0:1], in1=edk, op=ALU.mult)
            V.tensor_tensor(out=s4[:], in0=s1[:], in1=s4[:], op=ALU.subtract)
            V.tensor_scalar(out=s4[:], in0=s4[:], scalar1=2.0, scalar2=1e-30,
                            op0=ALU.mult, op1=ALU.max)
            V.tensor_tensor(out=vt[:, 0:1], in0=vt[:, 0:1], in1=edk,
                            op=ALU.subtract)
            V.reciprocal(s2[:], s4[:])    # 1/||v||^2 == beta/2
            asub = _sap(A2, (k + 1) * (N + 1), [ap2, B], [N, m], [1, m])
            V.tensor_tensor(
                out=omm[:, 0:m * m].rearrange("b (i j) -> b i j", i=m),
                in0=asub,
                in1=_sap(vvt, 0, [vstep, B], [0, m], [1, m]),
                op=ALU.mult)
            pvec = tmpm[:, 0:m]
            V.tensor_reduce(out=pvec,
                            in_=omm[:, 0:m * m].rearrange("b (i j) -> b i j",
                                                          i=m),
                            axis=AX.X, op=ALU.add)
            V.tensor_tensor(out=qvt[:, 0:m], in0=pvec, in1=vt, op=ALU.mult)
            V.tensor_reduce(out=s1[:], in_=qvt[:, 0:m], axis=AX.X, op=ALU.add)
            V.tensor_tensor(out=s1[:], in0=s1[:], in1=s2[:], op=ALU.mult)
            V.tensor_scalar(out=qvt[:, 0:m], in0=vt, scalar1=s1[:],
                            scalar2=None, op0=ALU.mult)
            V.tensor_tensor(out=qvt[:, 0:m], in0=pvec, in1=qvt[:, 0:m],
                            op=ALU.subtract)
            V.tensor_scalar(out=s4[:], in0=s2[:], scalar1=-2.0, scalar2=None,
                            op0=ALU.mult)    # -beta
            V.tensor_tensor(
                out=omm[:, 0:m * m].rearrange("b (i j) -> b i j", i=m),
                in0=_sap(vvt, 0, [vstep, B], [1, m], [0, m]),
                in1=_sap(qvt, 0, [qstep, B], [0, m], [1, m]),
                op=ALU.mult)
            V.scalar_tensor_tensor(
                out=asub,
                in0=omm[:, 0:m * m].rearrange("b (i j) -> b i j", i=m),
                scalar=s4[:], in1=asub, op0=ALU.mult, op1=ALU.add)
            V.tensor_tensor(
                out=omm[:, 0:m * m].rearrange("b (i j) -> b i j", i=m),
                in0=_sap(qvt, 0, [qstep, B], [1, m], [0, m]),
                in1=_sap(vvt, 0, [vstep, B], [0, m], [1, m]),
                op=ALU.mult)
            V.scalar_tensor_tensor(
                out=asub,
                in0=omm[:, 0:m * m].rearrange("b (i j) -> b i j", i=m),
                scalar=s4[:], in1=asub, op0=ALU.mult, op1=ALU.add)
        Td = sm.tile([B, N], F32)
        V.tensor_copy(Td[:], pdiag(A2))
        nege2 = sm.tile([B, N], F32)
        V.tensor_tensor(out=nege2[:], in0=Ed[:], in1=Ed[:], op=ALU.mult)
        V.tensor_scalar(out=nege2[:], in0=nege2[:], scalar1=-1.0,
                        scalar2=-1e-30, op0=ALU.mult, op1=ALU.add)
        if debug:
            tri = sm.tile([B, 2 * N], F32, name="dbtri")
            V.tensor_copy(tri[:, 0:N], Td[:])
            V.tensor_copy(tri[:, N:2 * N], Ed[:])
            nc.sync.dma_start(out=dbg["dbg_tri"][:], in_=tri[:])

        # --- Sturm multisection ---
        iotaF = sm.tile([B, NSHIFT], F32)
        ioi2 = sm.tile([B, NSHIFT], I32)
        nc.gpsimd.iota(ioi2[:], pattern=[[1, NSHIFT]], base=1,
                       channel_multiplier=0)
        V.tensor_copy(iotaF[:], ioi2[:])
        wid = sm.tile([B, 1], F32)
        V.tensor_tensor(out=wid[:], in0=hi_s[:], in1=lo_s[:], op=ALU.subtract)
        grid = sm.tile([B, NSHIFT], F32)
        dxm = sm.tile([B, N, NSHIFT], F32, tag="scr4k_a")
        pph = sm.tile([B, NSHIFT, N], F32)
        rr = sm.tile([B, NSHIFT], F32)
        cnt = sm.tile([B, NSHIFT], F32)
        stp = sm.tile([B, 1], F32)
        for it in range(NSTURM):
            V.tensor_scalar(out=stp[:], in0=wid[:],
                            scalar1=1.0 / (NSHIFT + 1.0), scalar2=None,
                            op0=ALU.mult)
            V.tensor_scalar(out=grid[:], in0=iotaF[:], scalar1=stp[:],
                            scalar2=lo_s[:], op0=ALU.mult, op1=ALU.add)
            V.tensor_tensor(out=dxm[:],
                            in0=_sap(Td, 0, [Td[:].ap[0][0], B], [1, N],
                                     [0, NSHIFT]),
                            in1=_sap(grid, 0, [grid[:].ap[0][0], B], [0, N],
                                     [1, NSHIFT]),
                            op=ALU.subtract)
            php = pph[:].ap[0][0]
            V.tensor_copy(_sap(pph, 0, [php, B], [N, NSHIFT]), dxm[:, 0, :])
            for i in range(1, N):
                V.reciprocal(rr[:], _sap(pph, i - 1, [php, B], [N, NSHIFT]))
                V.scalar_tensor_tensor(out=_sap(pph, i, [php, B], [N, NSHIFT]),
                                       in0=rr[:],
                                       scalar=nege2[:, i:i + 1],
                                       in1=dxm[:, i, :], op0=ALU.mult,
                                       op1=ALU.add)
            V.tensor_scalar(out=pph[:], in0=pph[:], scalar1=1e-25,
                            scalar2=None, op0=ALU.is_lt)
            V.tensor_reduce(out=cnt[:], in_=pph[:],
                            axis=AX.X, op=ALU.add)
            V.tensor_scalar(out=rr[:], in0=cnt[:], scalar1=0.0, scalar2=None,
                            op0=ALU.is_equal)
            V.tensor_reduce(out=s1[:], in_=rr[:], axis=AX.X, op=ALU.add)
            V.scalar_tensor_tensor(out=lo_s[:], in0=s1[:], scalar=stp[:],
                                   in1=lo_s[:], op0=ALU.mult, op1=ALU.add)
            V.tensor_copy(wid[:], stp[:])
        eigmin = sm.tile([B, 1], F32)
        V.tensor_scalar(out=eigmin[:], in0=wid[:], scalar1=0.5,
                        scalar2=None, op0=ALU.mult)
        V.tensor_tensor(out=eigmin[:], in0=lo_s[:], in1=eigmin[:], op=ALU.add)
        delta = sm.tile([B, 1], F32)
        V.tensor_scalar(out=delta[:], in0=eigmin[:], scalar1=-1.0,
                        scalar2=10.0, op0=ALU.mult, op1=ALU.add)
        if debug:
            de = sm.tile([B, 4], F32, name="dbeig")
            V.tensor_copy(de[:, 0:1], eigmin[:])
            V.tensor_copy(de[:, 1:2], delta[:])
            V.tensor_copy(de[:, 2:3], lo_s[:])
            V.tensor_copy(de[:, 3:4], hi_s[:])
            nc.sync.dma_start(out=dbg["dbg_eig"][:], in_=de[:])

        # ============ S6b: LDL^T of Prec + delta*I (sqrt-free) ============
        U = A2  # reuse A2 storage: overwrite with a fresh copy of prec
        V.tensor_copy(U[:], prec[:])
        V.tensor_scalar(out=pdiag(U), in0=pdiag(U), scalar1=delta[:],
                        scalar2=None, op0=ALU.add)
        dvec = sm.tile([B, N], F32)   # pivots d_k
        rvec = sm.tile([B, N], F32)   # 1/d_k
        for k in range(N):
            m = N - 1 - k
            dkk = _sap(U, k * (N + 1), [ap2, B], [1, 1])
            V.reciprocal(rvec[:, k:k + 1], dkk)
            if m > 0:
                urow = _sap(U, k * N + k + 1, [ap2, B], [1, m])
                V.tensor_scalar(out=vvt[:, 0:m], in0=urow, scalar1=-1.0,
                                scalar2=None, op0=ALU.mult)   # -a
                V.tensor_scalar(out=urow, in0=urow,
                                scalar1=rvec[:, k:k + 1], scalar2=None,
                                op0=ALU.mult)                 # l = a/d
                sub = _sap(U, (k + 1) * (N + 1), [ap2, B], [N, m], [1, m])
                V.tensor_tensor(
                    out=omm[:, 0:m * m].rearrange("b (i j) -> b i j", i=m),
                    in0=_sap(vvt, 0, [vstep, B], [1, m], [0, m]),
                    in1=_sap(U, k * N + k + 1, [ap2, B], [0, m], [1, m]),
                    op=ALU.mult)                  # (-a_i) * l_j
                V.tensor_tensor(
                    out=sub, in0=sub,
                    in1=omm[:, 0:m * m].rearrange("b (i j) -> b i j", i=m),
                    op=ALU.add)
        V.tensor_copy(dvec[:], pdiag(U))
        if debug:
            dbg_dump("dbg_chol", U[:])
        # logdet_loss = 0.5 * sum log d_k
        lud = sm.tile([B, N], F32)
        logdet = sm.tile([B, 1], F32)
        SC.activation(lud[:], dvec[:], ACTF.Ln, accum_out=logdet[:])
        V.tensor_scalar(out=logdet[:], in0=logdet[:], scalar1=0.5,
                        scalar2=None, op0=ALU.mult)

        # ========= S6c: M = (L^T)^{-1} (unit diag; XT[c,j] = M[j,c]) ======
        XT = per.tile([B, N * N], F32)
        V.memset(XT[:], 0.0)
        xtp = XT[:].ap[0][0]
        for k in range(N - 1, -1, -1):
            m = N - 1 - k
            if m > 0:
                V.tensor_tensor(
                    out=omm[:, 0:N * m].rearrange("b (c j) -> b c j", c=N),
                    in0=_sap(XT, k + 1, [xtp, B], [N, N], [1, m]),
                    in1=_sap(U, k * N + k + 1, [ap2, B], [0, N], [1, m]),
                    op=ALU.mult)
                V.tensor_reduce(
                    out=tmpm[:, 0:N],
                    in_=omm[:, 0:N * m].rearrange("b (c j) -> b c j", c=N),
                    axis=AX.X, op=ALU.add)
                V.tensor_scalar(out=_sap(XT, k, [xtp, B], [N, N]),
                                in0=tmpm[:, 0:N], scalar1=-1.0,
                                scalar2=None, op0=ALU.mult)
            V.tensor_scalar(out=_sap(XT, k * N + k, [xtp, B], [1, 1]),
                            in0=_sap(XT, k * N + k, [xtp, B], [1, 1]),
                            scalar1=1.0, scalar2=None, op0=ALU.add)
        if debug:
            dbg_dump("dbg_xinv", XT[:])
        # trinv = sum_c (sum_j M[j,c]^2) / d_c ; z_off = M.T... = U^-1 eps
        msq = sm.tile([B, N * N], F32, name="xsq", tag="esolv")
        V.tensor_tensor(out=msq[:], in0=XT[:], in1=XT[:], op=ALU.mult)
        V.tensor_reduce(out=tmpm[:, 0:N],
                        in_=msq[:].rearrange("b (c j) -> b c j", c=N),
                        axis=AX.X, op=ALU.add)
        trinv = sm.tile([B, 1], F32)
        V.tensor_tensor(out=qvt[:, 0:N], in0=tmpm[:, 0:N], in1=rvec[:],
                        op=ALU.mult)
        V.tensor_reduce(out=trinv[:], in_=qvt[:, 0:N], axis=AX.X, op=ALU.add)
        sqd = sm.tile([B, N], F32)
        SC.activation(sqd[:], rvec[:], ACTF.Sqrt)   # d^{-1/2}
        eh = sm.tile([B, N], F32)
        V.tensor_tensor(out=eh[:], in0=eps_sb[:], in1=sqd[:], op=ALU.mult)
        zoffm = sm.tile([B, N, N], F32, name="zoffm", tag="esolv")
        V.tensor_tensor(out=zoffm[:],
                        in0=_sap(XT, 0, [xtp, B], [1, N], [N, N]),
                        in1=_sap(eh, 0, [eh[:].ap[0][0], B], [0, N],
                                 [1, N]),
                        op=ALU.mult)
        z_off = sm.tile([B, N], F32)
        V.tensor_reduce(out=z_off[:], in_=zoffm[:], axis=AX.X, op=ALU.add)
        dbg_dump("dbg_zoff", z_off[:])
        z_samp = per.tile([B, N], F32R)
        V.tensor_tensor(out=z_samp[:], in0=z_b, in1=z_off[:], op=ALU.add)

        # latent_energy = 0.5*(|z*|^2 + trinv)
        zsq = sm.tile([B, N], F32, name="zsq")
        zn = sm.tile([B, 1], F32)
        SC.activation(zsq[:], z_b, ACTF.Square, accum_out=zn[:])
        lat = sm.tile([B, 1], F32)
        V.tensor_tensor(out=lat[:], in0=zn[:], in1=trinv[:], op=ALU.add)
        V.tensor_scalar(out=lat[:], in0=lat[:], scalar1=0.5, scalar2=None,
                        op0=ALU.mult)

        # ================= S5: recon at z_sample (d-layout) =================
        ps = psum_phase("ps5")
        zsT = per.tile([N, B], F32R)
        pe_transpose(zsT[:], z_samp[:], B, N)
        zsT_b16 = sm.tile([N, B], BF16)
        V.tensor_copy(zsT_b16[:], zsT[:].bitcast(F32))
        t2T = per.tile([128, KC_H, B], F8E4, tag="featA")
        for kq in range(4):
            pa2 = ps.tile([128, 4, B], F32, name="pa2", tag="pa2", bufs=2)
            for kk in range(4):
                kc = kq * 4 + kk
                nc.tensor.matmul(pa2[:, kk, :],
                                 db1r[0:1, kc * 128:(kc + 1) * 128],
                                 ones_row[0:1, 0:B], start=True, stop=False)
                nc.tensor.matmul(pa2[:, kk, :], w1dc_b16[:, kc, :],
                                 zsT_b16[:], start=False, stop=True)
            SC.activation(t2T[:, kq * 4:(kq + 1) * 4, :], pa2[:], ACTF.Tanh)
        pr22 = ps.tile([B, B], F32, name="pr22")
        for dc in range(KC_D):
            prc = ps.tile([128, B], F32, name="prc", tag="prc", bufs=2)
            for p in range(KC_H // 2):
                nc.tensor.matmul(prc[:],
                                 W2P8[p][:, :, dc * 128:(dc + 1) * 128],
                                 t2T[:, 2 * p:2 * p + 2, :],
                                 start=(p == 0), stop=(p == KC_H // 2 - 1),
                                 skip_group_check=(p not in
                                                   (0, KC_H // 2 - 1)),
                                 perf_mode=MMPM.DoubleRow)
            dfr = sm.tile([128, B], BF16, name="dfr", tag="diff", bufs=2)
            V.scalar_tensor_tensor(out=dfr[:], in0=prc[:],
                                   scalar=db2col[:, dc:dc + 1].bitcast(F32),
                                   in1=xT[:, dc, :].bitcast(F32),
                                   op0=ALU.add, op1=ALU.subtract)
            nc.tensor.matmul(pr22[:], dfr[:], dfr[:],
                             start=(dc == 0), stop=(dc == KC_D - 1),
                             skip_group_check=(dc not in (0, KC_D - 1)))
        dsq = sm.tile([B, B], F32)
        V.tensor_tensor(out=dsq[:], in0=pr22[:],
                        in1=ident[0:B, 0:B].bitcast(F32), op=ALU.mult)
        r2 = sm.tile([B, 1], F32)
        V.tensor_reduce(out=r2[:], in_=dsq[:], axis=AX.X, op=ALU.add)
        recon = sm.tile([B, 1], F32)
        V.scalar_tensor_tensor(out=recon[:], in0=r2[:], scalar=0.5,
                               in1=invsig2_b[:], op0=ALU.mult, op1=ALU.mult)

        # ================= outputs =================
        nlp = sm.tile([B, 1], F32)
        V.tensor_tensor(out=nlp[:], in0=recon[:], in1=lat[:], op=ALU.add)
        V.tensor_tensor(out=nlp[:], in0=nlp[:], in1=logdet[:], op=ALU.add)
        V.tensor_scalar(out=s1[:], in0=lsig[:], scalar1=float(D), scalar2=None,
                        op0=ALU.mult)
        V.tensor_tensor(out=nlp[:], in0=nlp[:], in1=s1[:], op=ALU.add)
        V.tensor_scalar(out=nlp[:], in0=nlp[:], scalar1=1.0 / D, scalar2=None,
                        op0=ALU.mult)
        outt = sm.tile([B, 5], F32)
        V.tensor_copy(outt[:, 0:1], nlp[:])
        V.tensor_copy(outt[:, 1:2], recon[:])
        V.tensor_copy(outt[:, 2:3], lat[:])
        V.tensor_copy(outt[:, 3:4], logdet[:])
        V.tensor_copy(outt[:, 4:5], sig_b)
        nc.sync.dma_start(out=out_d[:], in_=outt[:])
        psctx.close()

    return nc, dbg


MAX_LATENT_VAR = 0.1
_CACHE = {}


def _get_module(debug=False):
    key = bool(debug)
    if key not in _CACHE:
        nc, _ = build_module(debug)
        split_excess_waits(nc)
        _CACHE[key] = nc
    return _CACHE[key]


def kernel(**inputs):
    import ml_dtypes
    from concourse.bass_utils import run_bass_kernel_spmd
    nc = _get_module(False)
    x = np.asarray(inputs["x"], dtype=np.float32)
    eps = np.asarray(inputs["eps"], dtype=np.float32)
    rep = {k: np.asarray(v, dtype=np.float32) for k, v in inputs.items()
           if k not in ("x", "eps", "dec_W2", "enc_W1")}
    rep["enc_W1"] = np.ascontiguousarray(
        np.asarray(inputs["enc_W1"], dtype=np.float32)).astype(np.float16)
    w2 = np.ascontiguousarray(np.asarray(inputs["dec_W2"], dtype=np.float32))
    rep["dec_W2"] = w2.astype(ml_dtypes.float8_e4m3)
    rep["dec_W2T"] = np.ascontiguousarray(w2.T).astype(ml_dtypes.float8_e4m3)
    in_maps = []
    for c in range(NCORES):
        m = dict(rep)
        m["x"] = np.ascontiguousarray(x[c * B:(c + 1) * B])
        m["eps"] = np.ascontiguousarray(eps[0, c * B:(c + 1) * B, :])
        in_maps.append(m)
    r = run_bass_kernel_spmd(nc, in_maps, list(range(NCORES)))
    outs = np.concatenate([r.results[c]["out"] for c in range(NCORES)], axis=0)
    return (outs[:, 0], outs[:, 1], outs[:, 2], outs[:, 3], outs[:, 4])

